# revision 1
# baseline (speedup 1.0000x reference)
"""Trainium2 Bass kernel for nn_AutoIntTPPSameInfluence — PWL formulation.

dF(x) (the scalar derivative of the 1->64->64->64->1 tanh MLP) is a smooth
function of one variable, so the device evaluates a 127-knot piecewise-linear
least-squares fit  dF(x) ~= sum_m c_m * relu(x - k_m)  instead of the full
per-point MLP+JVP. Host computes the exact dF on a dense float64 grid, places
knots by curvature, and solves a weighted LSQ for c (end-to-end NLL error
~3e-5). Masked/padded pairs are packed with x = -50 (the ramp anchor), making
every relu feature exactly zero, so they contribute nothing.

Per 512-point unit on device:
  hx  = ones^T @ x_row          (K=1 f32r matmul: broadcast x to 128 parts)
  ft  = relu(hx - k)            (ACT, per-partition bias, fp32)
  red = segment-sum_16(ft)      (DVE reduce -> [128, 32])
  out = c^T @ red               (fp32 matmul -> [1, 32] seg partial dF sums)
The integral term F(T_END - t) uses the exact MLP forward pass (tiny).
Host: scatter seg sums to events, log/mask/reduce in float64.
"""

import numpy as np
from contextlib import ExitStack

import concourse.bass as bass
import concourse.bacc as bacc
import concourse.tile as tile
import concourse.mybir as mybir
from concourse.bass_utils import run_bass_kernel_spmd

B, L, H = 16, 320, 64
T_END = 100.0
NC = 8
C = 1024                   # points (columns) per unit
SEG = 16
SEGS_UNIT = C // SEG       # 32
FLUSH_U = 4
XMASK = np.float32(-50.0)  # ramp anchor; masked x -> all relu features 0
F32 = mybir.dt.float32
F32R = mybir.dt.float32r
F16 = mybir.dt.float16
Relu = mybir.ActivationFunctionType.Relu
Tanh = mybir.ActivationFunctionType.Tanh
Alu = mybir.AluOpType


def _pack(t, lens):
    """-> xrows [NC, Upc, C] f32, seg_ev [G], U."""
    bs, ks = [], []
    for b in range(B):
        n = int(lens[b])
        ksb = np.arange(1, n, dtype=np.int64)
        ks.append(ksb)
        bs.append(np.full_like(ksb, b))
    bs = np.concatenate(bs)
    ks = np.concatenate(ks)
    nseg = (ks + SEG - 1) // SEG
    total = int(nseg.sum())

    U = (total + SEGS_UNIT - 1) // SEGS_UNIT
    U = ((U + NC * FLUSH_U - 1) // (NC * FLUSH_U)) * (NC * FLUSH_U)
    G = U * SEGS_UNIT

    seg_b = np.zeros(G, dtype=np.int64)
    seg_k = np.zeros(G, dtype=np.int64)
    seg_j0 = np.zeros(G, dtype=np.int64)
    seg_ev = np.full(G, -1, dtype=np.int64)

    ev_idx = np.repeat(np.arange(len(ks)), nseg)
    seg_b[:total] = bs[ev_idx]
    seg_k[:total] = ks[ev_idx]
    seg_ev[:total] = seg_b[:total] * L + seg_k[:total]
    starts = np.concatenate([[0], np.cumsum(nseg)[:-1]])
    within = np.arange(total) - np.repeat(starts, nseg)
    seg_j0[:total] = within * SEG

    jj = seg_j0[:, None] + np.arange(SEG)[None, :]
    valid = jj < seg_k[:, None]
    jc = np.minimum(jj, L - 1)
    tj = t[seg_b[:, None], jc]
    tk = t[seg_b, seg_k][:, None]
    x = np.where(valid, (tk - tj).astype(np.float32), XMASK).astype(np.float32)

    xrows = x.reshape(U // 2, 2 * C)                   # 2 units per DMA batch
    hi = xrows.astype(np.float16)
    lo = (xrows - hi.astype(np.float32)).astype(np.float16)
    xhl = np.stack([hi, lo], axis=1)                   # [U/4, 2, 4C] fp16
    Upc = U // NC
    return np.ascontiguousarray(xhl.reshape(NC, Upc // 2, 2, 2 * C)), seg_ev, U


def _fit_pwl(W1, b1, W2, b2, W3, b3, W4):
    """127-knot weighted-LSQ relu fit of dF on [0,100]. float64 host math."""
    w1 = W1[:, 0].astype(np.float64)
    W2d, b2d = W2.astype(np.float64), b2.astype(np.float64)
    W3d, b3d = W3.astype(np.float64), b3.astype(np.float64)
    W4d = W4.astype(np.float64)
    b1d = b1.astype(np.float64)

    def dF(x):
        h1 = np.outer(w1, x) + b1d[:, None]
        a1 = np.tanh(h1)
        d1 = (1 - a1 ** 2) * w1[:, None]
        h2 = W2d @ a1 + b2d[:, None]
        a2 = np.tanh(h2)
        d2 = (1 - a2 ** 2) * (W2d @ d1)
        h3 = W3d @ a2 + b3d[:, None]
        a3 = np.tanh(h3)
        d3 = (1 - a3 ** 2) * (W3d @ d2)
        return (W4d @ d3)[0]

    gx = np.linspace(0.0, 100.0, 100001)
    gy = dF(gx)
    d2g = np.abs(np.gradient(np.gradient(gy, gx), gx))
    wgt = np.sqrt(d2g) + 1e-5
    cdf = np.cumsum(wgt)
    cdf /= cdf[-1]
    kp = np.interp(np.linspace(0, 1, 127), cdf, gx)
    kp[0] = 0.0
    kp[-1] = 100.0
    kp = np.unique(kp)
    kn = np.concatenate([[float(XMASK)], kp[:-1]])       # <=127 anchors
    A = np.maximum(gx[:, None] - kn[None, :], 0.0)
    wls = np.sqrt(100.01 - gx)
    c, *_ = np.linalg.lstsq(A * wls[:, None], gy * wls, rcond=None)
    # pad to exactly 128 partitions with dead features (k=large, c=0)
    pad = 128 - len(kn)
    kn = np.concatenate([kn, np.full(pad, 1e9)])
    c = np.concatenate([c, np.zeros(pad)])
    return kn.astype(np.float32), c.astype(np.float32)


def _consts(W1, b1, W2, b2, W3, b3, W4):
    w1 = W1[:, 0].astype(np.float32)
    zz = np.zeros((64, 64), np.float32)
    w2blk = np.block([[W2.T, zz], [zz, W2.T]]).astype(np.float32)
    w3blk = np.block([[W3.T, zz], [zz, W3.T]]).astype(np.float32)
    w4 = W4[0].astype(np.float32)
    b123 = np.stack([np.tile(b1, 2), np.tile(b2, 2), np.tile(b3, 2)],
                    axis=1).astype(np.float32)
    gw1 = np.zeros((2, 128), np.float32)
    gw1[0, :64] = w1
    gw1[1, 64:] = w1
    gw4 = np.zeros((128, 2), np.float32)
    gw4[:64, 0] = w4
    gw4[64:, 1] = w4
    kn, c = _fit_pwl(W1, b1, W2, b2, W3, b3, W4)

    # f32r consts [128, 388]: w2blk | w3blk | gw4 2 | gw1 rows0:2 cols258:386
    # | ones lhsT row0 col 386:387.. pack ones as [1,128] rows0 cols 260:388
    constsR = np.zeros((128, 516), np.float32)
    constsR[:, 0:128] = w2blk
    constsR[:, 128:256] = w3blk
    constsR[:, 256:258] = gw4
    constsR[0:2, 258:386] = gw1
    # cols 388:516 unused (x broadcast now uses the fp16 ones16 const)
    # f32 consts [128, 6]: b123 | negk | c
    constsF = np.concatenate(
        [b123, (-kn).reshape(128, 1), c.reshape(128, 1)], axis=1)
    ones16 = np.ones((2, 128), np.float16)
    return dict(constsR=constsR, constsF=np.ascontiguousarray(constsF),
                ones16=ones16)


_PROGRAM_CACHE = {}


def build_program(Upc):
    if Upc in _PROGRAM_CACHE:
        return _PROGRAM_CACHE[Upc]
    NF = Upc // FLUSH_U
    nc = bacc.Bacc("TRN2", target_bir_lowering=False, debug=False,
                   enable_asserts=False)

    xr_d = nc.dram_tensor("xrows", [Upc // 2, 2, 2 * C], F16,
                          kind="ExternalInput")
    ones16_d = nc.dram_tensor("ones16", [2, 128], F16, kind="ExternalInput")
    constsR_d = nc.dram_tensor("constsR", [128, 516], F32R, kind="ExternalInput")
    constsF_d = nc.dram_tensor("constsF", [128, 5], F32, kind="ExternalInput")
    grhs_d = nc.dram_tensor("grhs", [2, L], F32R, kind="ExternalInput")
    outs_d = nc.dram_tensor("out_s", [NF, 1, FLUSH_U * SEGS_UNIT], F32,
                            kind="ExternalOutput")
    outg_d = nc.dram_tensor("out_g", [2, L], F32, kind="ExternalOutput")

    with tile.TileContext(nc) as tc, ExitStack() as ctx, \
            nc.allow_low_precision(reason="float32r shares float32 bit layout"):
        consts = ctx.enter_context(tc.tile_pool(name="consts", bufs=1))
        xr_p = ctx.enter_context(tc.tile_pool(name="xr", bufs=4))
        ft_p = ctx.enter_context(tc.tile_pool(name="ft", bufs=3))
        hxb_p = ctx.enter_context(tc.tile_pool(name="hxb", bufs=3))
        red_p = ctx.enter_context(tc.tile_pool(name="red", bufs=3))
        stage_p = ctx.enter_context(tc.tile_pool(name="stage", bufs=2))
        hx_p = ctx.enter_context(tc.tile_pool(name="hx", bufs=2, space="PSUM"))
        pf_p = ctx.enter_context(tc.tile_pool(name="pf", bufs=2, space="PSUM"))
        gact_p = ctx.enter_context(tc.tile_pool(name="gact", bufs=2))

        cR_raw = consts.tile([128, 516], F32R, tag="cRraw")
        nc.sync.dma_start(out=cR_raw[:], in_=constsR_d.ap())
        cR = consts.tile([128, 516], F32R, tag="cR")
        nc.scalar.copy(cR[:], cR_raw[:])
        cF_raw = consts.tile([128, 5], F32, tag="cFraw")
        nc.sync.dma_start(out=cF_raw[:], in_=constsF_d.ap())
        cF = consts.tile([128, 5], F32, tag="cF")
        nc.vector.tensor_copy(cF[:], cF_raw[:])
        w2blk_t = cR[:, 0:128]
        w3blk_t = cR[:, 128:256]
        gw4_t = cR[:, 256:258]
        gw1_t = cR[0:2, 258:386]
        ones_t = cR[0:1, 388:516]
        b123_t = cF[:, 0:3]
        negk_t = cF[:, 3:4]
        c_t = cF[:, 4:5]
        o16_raw = consts.tile([2, 128], F16, tag="o16raw")
        nc.sync.dma_start(out=o16_raw[:], in_=ones16_d.ap())
        ones16_t = consts.tile([2, 128], F16, tag="o16")
        nc.scalar.copy(ones16_t[:], o16_raw[:])
        grhs_raw = xr_p.tile([2, L], F32R, tag="grhsraw")
        nc.sync.dma_start(out=grhs_raw[:], in_=grhs_d.ap())
        grhs_t = xr_p.tile([2, L], F32R, tag="grhs")
        nc.vector.tensor_copy(grhs_t[:], grhs_raw[:])

        # ---- main PWL loop ----
        # x rows arrive as fp16 (hi, lo) pairs: K=2 ones-matmul reconstructs
        # x exactly in the fp32 PSUM accumulate at full 1 cyc/row rate.
        DMA_U = 2                    # units per x DMA batch
        xrb_t = None
        redb_t = None
        for u in range(Upc):
            if u % DMA_U == 0:
                xrb_t = xr_p.tile([2, DMA_U * C], F16, tag="xrb")
                nc.sync.dma_start(out=xrb_t[:], in_=xr_d.ap()[u // DMA_U])
            r = u % DMA_U

            hx = hx_p.tile([128, C], F32, tag="hx")
            for h0 in range(0, C, 512):
                nc.tensor.matmul(out=hx[:, h0:h0 + 512], lhsT=ones16_t[:],
                                 rhs=xrb_t[:, r * C + h0:r * C + h0 + 512],
                                 start=True, stop=True)
            ft = ft_p.tile([128, C], F32, tag="ft")
            nc.scalar.activation(ft[:], hx[:], Relu, bias=negk_t[:])
            if u % FLUSH_U == 0:
                redb_t = red_p.tile([128, FLUSH_U * SEGS_UNIT], F32, tag="red")
            s0 = (u % FLUSH_U) * SEGS_UNIT
            nc.vector.tensor_reduce(
                out=redb_t[:, s0:s0 + SEGS_UNIT],
                in_=ft[:].rearrange("p (s d) -> p s d", d=SEG),
                axis=mybir.AxisListType.X, op=Alu.add)

            if u % FLUSH_U == FLUSH_U - 1:
                pf_t = pf_p.tile([1, FLUSH_U * SEGS_UNIT], F32, tag="pf")
                nc.tensor.matmul(out=pf_t[:], lhsT=c_t[:], rhs=redb_t[:],
                                 start=True, stop=True)
                stage_t = stage_p.tile([1, FLUSH_U * SEGS_UNIT], F32, tag="st")
                nc.scalar.copy(stage_t[:], pf_t[:])
                nc.sync.dma_start(out=outs_d.ap()[u // FLUSH_U],
                                  in_=stage_t[:])

        # ---- exact-MLP G pass (integral term), 2 batch rows ----
        gh1 = hx_p.tile([128, L], F32, tag="hx")
        nc.tensor.matmul(out=gh1[:], lhsT=gw1_t[:], rhs=grhs_t[:],
                         start=True, stop=True)
        ga1 = gact_p.tile([128, L], F32R, tag="ga")
        nc.scalar.activation(ga1[:], gh1[:], Tanh, bias=b123_t[:, 0:1])
        gh2 = hx_p.tile([128, L], F32, tag="hx")
        nc.tensor.matmul(out=gh2[:], lhsT=w2blk_t[:], rhs=ga1[:],
                         start=True, stop=True)
        ga2 = gact_p.tile([128, L], F32R, tag="ga")
        nc.scalar.activation(ga2[:], gh2[:], Tanh, bias=b123_t[:, 1:2])
        gh3 = hx_p.tile([128, L], F32, tag="hx")
        nc.tensor.matmul(out=gh3[:], lhsT=w3blk_t[:], rhs=ga2[:],
                         start=True, stop=True)
        ga3 = gact_p.tile([128, L], F32R, tag="ga")
        nc.scalar.activation(ga3[:], gh3[:], Tanh, bias=b123_t[:, 2:3])
        gout = pf_p.tile([2, L], F32, tag="gout")
        nc.tensor.matmul(out=gout[:], lhsT=gw4_t[:], rhs=ga3[:],
                         start=True, stop=True)
        gstage = stage_p.tile([2, L], F32, tag="gstage")
        nc.scalar.copy(gstage[:], gout[:])
        nc.sync.dma_start(out=outg_d.ap(), in_=gstage[:])

    nc.compile()
    prog = (nc, Upc)
    _PROGRAM_CACHE[Upc] = prog
    return prog


def kernel(seq_pads, background, W1, b1, W2, b2, W3, b3, W4, b4, seq_lens):
    t = np.asarray(seq_pads)[:, :, 0].astype(np.float32)
    lens = np.asarray(seq_lens).astype(np.int64)
    xrows, seg_ev, U = _pack(t, lens)
    cs = _consts(np.asarray(W1, np.float32), np.asarray(b1, np.float32),
                 np.asarray(W2, np.float32), np.asarray(b2, np.float32),
                 np.asarray(W3, np.float32), np.asarray(b3, np.float32),
                 np.asarray(W4, np.float32))
    Upc = U // NC
    nc, _ = build_program(Upc)

    in_maps = []
    for c in range(NC):
        m = dict(cs)
        m["xrows"] = xrows[c]
        m["grhs"] = np.ascontiguousarray(
            (T_END - t[2 * c:2 * c + 2]).astype(np.float32))
        in_maps.append(m)

    res = run_bass_kernel_spmd(nc, in_maps, core_ids=list(range(NC))).results
    if any(not np.isfinite(res[c][k]).all() for c in range(NC)
           for k in ("out_s", "out_g")):
        res = run_bass_kernel_spmd(nc, in_maps, core_ids=list(range(NC))).results

    parts = []
    Gmat = np.zeros((B, L), np.float64)
    for c in range(NC):
        parts.append(res[c]["out_s"].reshape(-1))        # [Upc*32] in seg order
        Gmat[2 * c:2 * c + 2] = res[c]["out_g"]
    partials = np.concatenate(parts)

    S = np.zeros(B * L, np.float64)
    ok = seg_ev >= 0
    np.add.at(S, seg_ev[ok], partials[ok].astype(np.float64))
    S = S.reshape(B, L)

    bg = float(np.asarray(background)[0])
    lam = bg + S
    mask = np.arange(L)[None, :] < lens[:, None]
    sum_log = np.log(np.where(mask, lam, 1.0)).sum()

    h = np.tanh(np.asarray(b1, np.float64))
    h = np.tanh(np.asarray(W2, np.float64) @ h + np.asarray(b2, np.float64))
    h = np.tanh(np.asarray(W3, np.float64) @ h + np.asarray(b3, np.float64))
    F0 = float((np.asarray(W4, np.float64) @ h + np.asarray(b4, np.float64))[0])

    b4f = float(np.asarray(b4)[0])
    ints = np.where(mask, Gmat + b4f - F0, 0.0).sum(axis=1) + T_END * bg
    nll = -(sum_log - ints.sum()) / B
    return np.float32(nll)



# revision 3
# speedup vs baseline: 5.7686x; 5.7686x over previous
"""Trainium2 Bass kernel for nn_AutoIntTPPSameInfluence — head/tail PWL split.

dF(x) (scalar derivative of the 1->64->64->64->1 tanh MLP) decays four orders
of magnitude within x < ~2.5 and is glass-smooth beyond.  The kernel exploits
this:

  tail (x >= XC):  dF is fit by per-zone cubics (6 log-spaced zones).  Sums of
      a cubic over a contiguous j-range reduce to prefix-sum moments of t —
      the host aggregates these exactly in float64 (O(B*L) work, no per-pair
      math).
  head (x < XC):   all curvature lives here (~29K pairs of the 460K total).
      The device evaluates a 14-knot relu PWL per point via the baseline's
      relu-feature pipeline: ones-matmul broadcast -> ACT relu with
      per-partition knot bias -> DVE segment reduce (SEG=4) -> coefficient
      matmul.  8 independent streams (one per 16-partition block) pack 8
      points per column, so every engine does 8x less work per point.
      The affine component of the head fit is host-aggregated like the tail.

The integral term F(T_END - t_k) gets the identical treatment (shared knots,
second coefficient column per stream), removing the exact-MLP pass entirely.
Fit weights come from the empirical x/y histograms, which drives the
end-to-end NLL error to ~1e-5 (tolerance 2e-2).
"""

import numpy as np
from contextlib import ExitStack

import concourse.bass as bass
import concourse.bacc as bacc
import concourse.tile as tile
import concourse.mybir as mybir
from concourse.bass_utils import run_bass_kernel_spmd

B, L, H = 16, 320, 64
T_END = 100.0
NC = 8
P = 8                    # streams = partition blocks of 16
BLK = 128 // P           # 16 partitions per stream
M = 14                   # live knots per stream (<= BLK)
SEG = 4                  # points per segment
XC = 2.5                 # head/tail split
NZ = 6                   # tail zones
DEG = 3                  # tail polynomial degree
COLG = 64                # column-count granularity per core
F32 = mybir.dt.float32
F16 = mybir.dt.float16
Relu = mybir.ActivationFunctionType.Relu
Alu = mybir.AluOpType

_BREAKS = XC * (100.0 / XC) ** (np.arange(NZ + 1) / NZ)
_BREAKS[-1] = 100.0001


# ---------------------------------------------------------------- MLP (host)
def _mk_fns(W1, b1, W2, b2, W3, b3, W4, b4):
    w1 = W1[:, 0]

    def dF(x):
        x = np.asarray(x, np.float64)
        h1 = np.multiply.outer(w1, x) + b1[:, None]
        a1 = np.tanh(h1)
        d1 = (1 - a1 ** 2) * w1[:, None]
        h2 = W2 @ a1 + b2[:, None]
        a2 = np.tanh(h2)
        d2 = (1 - a2 ** 2) * (W2 @ d1)
        h3 = W3 @ a2 + b3[:, None]
        a3 = np.tanh(h3)
        d3 = (1 - a3 ** 2) * (W3 @ d2)
        return (W4 @ d3)[0]

    def F(x):
        x = np.asarray(x, np.float64)
        h1 = np.tanh(np.multiply.outer(w1, x) + b1[:, None])
        h2 = np.tanh(W2 @ h1 + b2[:, None])
        h3 = np.tanh(W3 @ h2 + b3[:, None])
        return (W4 @ h3)[0] + b4[0]

    return dF, F


# ------------------------------------------------------------------ fits
def _fits(dF, F, t, lens):
    """Zone cubics + shared-knot head PWLs, weighted by empirical densities."""
    mask = np.arange(L)[None, :] < lens[:, None]
    # all pair diffs of log-events (for zone weights); O(B*L^2) floats, ~20ms
    allx = []
    for b in range(B):
        n = int(lens[b])
        d = t[b, :n, None] - t[b, None, :n]
        allx.append(d[np.tril_indices(n, -1)])
    allx = np.concatenate(allx)
    ally = (T_END - t)[mask]

    def zonefits(fn, data):
        cfs, mids = [], []
        for z in range(NZ):
            lo, hi = _BREAKS[z], _BREAKS[z + 1]
            gx = np.linspace(lo, hi, 4001)
            mid = 0.5 * (lo + hi)
            mids.append(mid)
            V = np.vander(gx - mid, DEG + 1, increasing=True)
            hw, be = np.histogram(data[(data >= lo) & (data < hi)],
                                  bins=80, range=(lo, hi))
            w = np.sqrt(np.interp(gx, 0.5 * (be[:-1] + be[1:]),
                                  hw.astype(np.float64)) + 1.0)
            cf, *_ = np.linalg.lstsq(V * w[:, None], fn(gx) * w, rcond=None)
            cfs.append(cf)
        return np.array(cfs), np.array(mids)

    cQ, midQ = zonefits(dF, allx)
    cQF, midQF = zonefits(F, ally)

    # shared knots on [0, XC] from blended curvature
    gx = np.linspace(0.0, XC, 40001)
    gyd = dF(gx)
    gyF = F(gx)
    d2d = np.abs(np.gradient(np.gradient(gyd, gx), gx))
    d2F = np.abs(np.gradient(np.gradient(gyF, gx), gx))
    wk = np.sqrt(d2d / max(np.abs(gyd).mean(), 1e-9) + 3.0 * d2F) + 1e-6
    cdf = np.cumsum(wk)
    cdf /= cdf[-1]
    kn = np.unique(np.interp(np.linspace(0, 1, M + 2)[1:-1], cdf, gx))
    kn = np.clip(kn, 1e-4, None)
    feats = np.maximum(gx[:, None] - kn[None, :], 0.0)
    A = np.concatenate([np.ones_like(gx)[:, None], gx[:, None], feats], 1)

    def headfit(gy, data):
        hw, be = np.histogram(data, bins=100, range=(0, XC))
        w = np.sqrt(np.interp(gx, 0.5 * (be[:-1] + be[1:]),
                              hw.astype(np.float64)) + 2.0)
        cf, *_ = np.linalg.lstsq(A * w[:, None], gy * w, rcond=None)
        return cf

    hx = allx[allx < XC]
    hy = ally[ally < XC]
    cfd = headfit(gyd, hx)
    cfF = headfit(gyF, hy)
    return cQ, midQ, cQF, midQF, kn, cfd, cfF


# ------------------------------------------------------------------ packing
def _pack(t, lens, kn):
    """Head points -> [NC, P, COLS] fp16 + seg target map + host-side sums'
    raw material (per-event head ranges)."""
    nk = len(kn)
    xs_all, tgt_all = [], []
    head_cnt = np.zeros((B, L), np.int64)      # h_i
    head_sum = np.zeros((B, L), np.float64)    # sum of head x per event
    for b in range(B):
        tb = t[b]
        n = int(lens[b])
        j0 = np.minimum(np.searchsorted(tb, tb - XC, side='right'),
                        np.arange(L))
        for i in range(1, n):
            h = i - j0[i]
            if h == 0:
                continue
            x = tb[i] - tb[j0[i]:i]
            head_cnt[b, i] = h
            head_sum[b, i] = x.sum()
            pad = (-h) % SEG
            if pad:
                x = np.concatenate([x, np.zeros(pad)])
            xs_all.append(x)
            tgt_all.append(np.full(len(x) // SEG, b * L + i, np.int64))
        # F-head points for the integral term
        y = T_END - tb[:n]
        yh = y[y < XC]
        if len(yh):
            pad = (-len(yh)) % SEG
            if pad:
                yh = np.concatenate([yh, np.zeros(pad)])
            xs_all.append(yh)
            tgt_all.append(np.full(len(yh) // SEG, B * L + b, np.int64))
    xs = np.concatenate(xs_all)
    tgt = np.concatenate(tgt_all)
    gseg = len(tgt)
    # pad segs to NC * P * (COLS/SEG), COLS multiple of COLG
    cols = -(-gseg * SEG // (NC * P * COLG)) * COLG
    cap = NC * P * (cols // SEG)
    xs = np.concatenate([xs, np.zeros((cap - gseg) * SEG)])
    tgt = np.concatenate([tgt, np.full(cap - gseg, -1, np.int64)])
    xr = np.ascontiguousarray(
        xs.astype(np.float16).reshape(NC, P, cols))
    return xr, tgt.reshape(NC, P, cols // SEG), cols, head_cnt, head_sum


def _consts(kn, cfd, cfF):
    nk = len(kn)
    negk = np.full(128, -1e9, np.float64)
    cmat = np.zeros((128, 2 * P), np.float64)
    for r in range(P):
        negk[BLK * r:BLK * r + nk] = -kn
        cmat[BLK * r:BLK * r + nk, 2 * r] = cfd[2:]
        cmat[BLK * r:BLK * r + nk, 2 * r + 1] = cfF[2:]
    consts = np.concatenate([negk[:, None], cmat], 1).astype(np.float32)
    ones8 = np.zeros((P, 128), np.float16)
    for r in range(P):
        ones8[r, BLK * r:BLK * (r + 1)] = 1.0
    return np.ascontiguousarray(consts), np.ascontiguousarray(ones8)


# ------------------------------------------------------------ host tail sums
def _host_sums(t, lens, cQ, midQ, cQF, midQF, cfd, cfF, head_cnt, head_sum):
    """per-event tail-zone + head-affine sums, and integral-term host part."""
    host_pe = np.zeros((B, L))
    host_int = np.zeros(B)
    iota = np.arange(L)
    for b in range(B):
        tb = t[b]
        n = int(lens[b])
        S = [np.concatenate([[0.0], np.cumsum(tb ** d)]) for d in range(DEG + 1)]
        acc = np.zeros(L)
        for z in range(NZ):
            lo, hi = _BREAKS[z], _BREAKS[z + 1]
            j0 = np.minimum(np.searchsorted(tb, tb - hi, side='right'), iota)
            j1 = np.minimum(np.searchsorted(tb, tb - lo, side='right'), iota)
            m0 = (j1 - j0).astype(np.float64)
            s1 = S[1][j1] - S[1][j0]
            s2 = S[2][j1] - S[2][j0]
            s3 = S[3][j1] - S[3][j0]
            u = tb - midQ[z]
            m1 = u * m0 - s1
            m2 = u * u * m0 - 2 * u * s1 + s2
            m3 = u ** 3 * m0 - 3 * u * u * s1 + 3 * u * s2 - s3
            acc += cQ[z, 0] * m0 + cQ[z, 1] * m1 + cQ[z, 2] * m2 + cQ[z, 3] * m3
        # head affine part
        acc += cfd[0] * head_cnt[b] + cfd[1] * head_sum[b]
        host_pe[b] = acc
        # integral term: direct per-event zone cubic + head affine
        y = T_END - tb[:n]
        q = 0.0
        for z in range(NZ):
            sel = (y >= _BREAKS[z]) & (y < _BREAKS[z + 1])
            if sel.any():
                yz = y[sel] - midQF[z]
                q += sum(cQF[z, d] * (yz ** d).sum() for d in range(DEG + 1))
        yh = y[y < XC]
        q += cfF[0] * len(yh) + cfF[1] * yh.sum()
        host_int[b] = q
    return host_pe, host_int


# ------------------------------------------------------------------ program
_PROGRAM_CACHE = {}


def build_program(cols):
    if cols in _PROGRAM_CACHE:
        return _PROGRAM_CACHE[cols]
    spc = cols // SEG
    nc = bacc.Bacc("TRN2", target_bir_lowering=False, debug=False,
                   enable_asserts=False)
    xr_d = nc.dram_tensor("xr", [P, cols], F16, kind="ExternalInput")
    ones_d = nc.dram_tensor("ones8", [P, 128], F16, kind="ExternalInput")
    consts_d = nc.dram_tensor("consts", [128, 1 + 2 * P], F32,
                              kind="ExternalInput")
    out_d = nc.dram_tensor("out", [2 * P, spc], F32, kind="ExternalOutput")

    with tile.TileContext(nc) as tc, ExitStack() as ctx:
        cons = ctx.enter_context(tc.tile_pool(name="cons", bufs=1))
        xr_p = ctx.enter_context(tc.tile_pool(name="xr", bufs=1))
        ft_p = ctx.enter_context(tc.tile_pool(name="ft", bufs=2))
        red_p = ctx.enter_context(tc.tile_pool(name="red", bufs=1))
        st_p = ctx.enter_context(tc.tile_pool(name="st", bufs=1))
        hx_p = ctx.enter_context(tc.tile_pool(name="hx", bufs=2, space="PSUM"))
        po_p = ctx.enter_context(tc.tile_pool(name="po", bufs=1, space="PSUM"))

        cF = cons.tile([128, 1 + 2 * P], F32, tag="cF")
        nc.scalar.dma_start(out=cF[:], in_=consts_d.ap())
        o16 = cons.tile([P, 128], F16, tag="o16")
        nc.gpsimd.dma_start(out=o16[:], in_=ones_d.ap())
        xr_t = xr_p.tile([P, cols], F16, tag="xr")
        nc.sync.dma_start(out=xr_t[:], in_=xr_d.ap())
        negk = cF[:, 0:1]
        cmat = cF[:, 1:1 + 2 * P]

        red_t = red_p.tile([128, spc], F32, tag="red")
        for c0 in range(0, cols, 512):
            cw = min(512, cols - c0)
            hx = hx_p.tile([128, cw], F32, tag="hx")
            nc.tensor.matmul(out=hx[:], lhsT=o16[:],
                             rhs=xr_t[:, c0:c0 + cw], start=True, stop=True)
            ft = ft_p.tile([128, cw], F32, tag="ft")
            nc.scalar.activation(ft[:], hx[:], Relu, bias=negk)
            nc.vector.tensor_reduce(
                out=red_t[:, c0 // SEG:(c0 + cw) // SEG],
                in_=ft[:].rearrange("p (s d) -> p s d", d=SEG),
                axis=mybir.AxisListType.X, op=Alu.add)

        po = po_p.tile([2 * P, spc], F32, tag="po")
        nc.tensor.matmul(out=po[:], lhsT=cmat, rhs=red_t[:],
                         start=True, stop=True)
        st = st_p.tile([2 * P, spc], F32, tag="st")
        nc.scalar.copy(st[:], po[:])
        nc.sync.dma_start(out=out_d.ap(), in_=st[:])

    nc.compile()
    prog = (nc, cols)
    _PROGRAM_CACHE[cols] = prog
    return prog


# ------------------------------------------------------------------ driver
def _build_all(seq_pads, background, W1, b1, W2, b2, W3, b3, W4, b4, seq_lens):
    t = np.asarray(seq_pads, np.float64)[:, :, 0]
    lens = np.asarray(seq_lens).astype(np.int64)
    f64 = lambda a: np.asarray(a, np.float64)
    dF, F = _mk_fns(f64(W1), f64(b1), f64(W2), f64(b2), f64(W3), f64(b3),
                    f64(W4), f64(b4))
    cQ, midQ, cQF, midQF, kn, cfd, cfF = _fits(dF, F, t, lens)
    xr, tgt, cols, head_cnt, head_sum = _pack(t, lens, kn)
    consts, ones8 = _consts(kn, cfd, cfF)
    host_pe, host_int = _host_sums(t, lens, cQ, midQ, cQF, midQF, cfd, cfF,
                                   head_cnt, head_sum)
    nc, _ = build_program(cols)
    in_maps = [dict(xr=xr[c], ones8=ones8, consts=consts) for c in range(NC)]

    # F(0) and mask bookkeeping for the finalizer
    h = np.tanh(f64(b1))
    h = np.tanh(f64(W2) @ h + f64(b2))
    h = np.tanh(f64(W3) @ h + f64(b3))
    F0 = float((f64(W4) @ h + f64(b4))[0])
    bg = float(np.asarray(background)[0])
    mask = np.arange(L)[None, :] < lens[:, None]

    def finish(results):
        pe = host_pe.copy().reshape(-1)
        ints = host_int.copy()
        spc = cols // SEG
        for c in range(NC):
            o = np.asarray(results[c]["out"], np.float64)   # [2P, spc]
            for r in range(P):
                tg = tgt[c, r]
                ev = tg[(tg >= 0) & (tg < B * L)]
                np.add.at(pe, ev, o[2 * r][(tg >= 0) & (tg < B * L)])
                fb = tg[tg >= B * L]
                np.add.at(ints, fb - B * L, o[2 * r + 1][tg >= B * L])
        pe = pe.reshape(B, L)
        lam = bg + pe
        sum_log = np.where(mask, np.log(np.where(mask & (lam > 0), lam, 1.0)),
                           0.0).sum()
        ints_full = ints - mask.sum(1) * F0 + T_END * bg
        nll = -(sum_log - ints_full.sum()) / B
        return np.float32(nll)

    return nc, in_maps, finish


def kernel(seq_pads, background, W1, b1, W2, b2, W3, b3, W4, b4, seq_lens):
    nc, in_maps, finish = _build_all(seq_pads, background, W1, b1, W2, b2,
                                     W3, b3, W4, b4, seq_lens)
    res = run_bass_kernel_spmd(nc, in_maps, core_ids=list(range(NC))).results
    if any(not np.isfinite(res[c]["out"]).all() for c in range(NC)):
        res = run_bass_kernel_spmd(nc, in_maps,
                                   core_ids=list(range(NC))).results
    return finish(res)


# revision 9
# speedup vs baseline: 6.1538x; 1.0668x over previous
"""Trainium2 Bass kernel for nn_AutoIntTPPSameInfluence — head/tail PWL split.

dF(x) (scalar derivative of the 1->64->64->64->1 tanh MLP) decays four orders
of magnitude within x < ~2.5 and is glass-smooth beyond.  The kernel exploits
this:

  tail (x >= XC):  dF is fit by per-zone cubics (6 log-spaced zones).  Sums of
      a cubic over a contiguous j-range reduce to prefix-sum moments of t —
      the host aggregates these exactly in float64 (O(B*L) work, no per-pair
      math).
  head (x < XC):   all curvature lives here (~29K pairs of the 460K total).
      The device evaluates a 14-knot relu PWL per point via the baseline's
      relu-feature pipeline: ones-matmul broadcast -> ACT relu with
      per-partition knot bias -> DVE segment reduce (SEG=4) -> coefficient
      matmul.  8 independent streams (one per 16-partition block) pack 8
      points per column, so every engine does 8x less work per point.
      The affine component of the head fit is host-aggregated like the tail.

The integral term F(T_END - t_k) gets the identical treatment (shared knots,
second coefficient column per stream), removing the exact-MLP pass entirely.
Fit weights come from the empirical x/y histograms, which drives the
end-to-end NLL error to ~1e-5 (tolerance 2e-2).
"""

import numpy as np
from contextlib import ExitStack

import concourse.bass as bass
import concourse.bacc as bacc
import concourse.tile as tile
import concourse.mybir as mybir
from concourse.bass_utils import run_bass_kernel_spmd

B, L, H = 16, 320, 64
T_END = 100.0
NC = 8
P = 8                    # streams = partition blocks of 16
BLK = 128 // P           # 16 partitions per stream
M = 14                   # live knots per stream (<= BLK)
SEG = 4                  # points per segment
XC = 2.5                 # head/tail split
NZ = 6                   # tail zones
DEG = 3                  # tail polynomial degree
COLG = 64                # column-count granularity per core
F32 = mybir.dt.float32
F16 = mybir.dt.float16
Relu = mybir.ActivationFunctionType.Relu
Alu = mybir.AluOpType

_BREAKS = XC * (100.0 / XC) ** (np.arange(NZ + 1) / NZ)
_BREAKS[-1] = 100.0001


# ---------------------------------------------------------------- MLP (host)
def _mk_fns(W1, b1, W2, b2, W3, b3, W4, b4):
    w1 = W1[:, 0]

    def dF(x):
        x = np.asarray(x, np.float64)
        h1 = np.multiply.outer(w1, x) + b1[:, None]
        a1 = np.tanh(h1)
        d1 = (1 - a1 ** 2) * w1[:, None]
        h2 = W2 @ a1 + b2[:, None]
        a2 = np.tanh(h2)
        d2 = (1 - a2 ** 2) * (W2 @ d1)
        h3 = W3 @ a2 + b3[:, None]
        a3 = np.tanh(h3)
        d3 = (1 - a3 ** 2) * (W3 @ d2)
        return (W4 @ d3)[0]

    def F(x):
        x = np.asarray(x, np.float64)
        h1 = np.tanh(np.multiply.outer(w1, x) + b1[:, None])
        h2 = np.tanh(W2 @ h1 + b2[:, None])
        h3 = np.tanh(W3 @ h2 + b3[:, None])
        return (W4 @ h3)[0] + b4[0]

    return dF, F


# ------------------------------------------------------------------ fits
def _fits(dF, F, t, lens):
    """Zone cubics + shared-knot head PWLs, weighted by empirical densities."""
    mask = np.arange(L)[None, :] < lens[:, None]
    # all pair diffs of log-events (for zone weights); O(B*L^2) floats, ~20ms
    allx = []
    for b in range(B):
        n = int(lens[b])
        d = t[b, :n, None] - t[b, None, :n]
        allx.append(d[np.tril_indices(n, -1)])
    allx = np.concatenate(allx)
    ally = (T_END - t)[mask]

    def zonefits(fn, data):
        cfs, mids = [], []
        for z in range(NZ):
            lo, hi = _BREAKS[z], _BREAKS[z + 1]
            gx = np.linspace(lo, hi, 4001)
            mid = 0.5 * (lo + hi)
            mids.append(mid)
            V = np.vander(gx - mid, DEG + 1, increasing=True)
            hw, be = np.histogram(data[(data >= lo) & (data < hi)],
                                  bins=80, range=(lo, hi))
            w = np.sqrt(np.interp(gx, 0.5 * (be[:-1] + be[1:]),
                                  hw.astype(np.float64)) + 1.0)
            cf, *_ = np.linalg.lstsq(V * w[:, None], fn(gx) * w, rcond=None)
            cfs.append(cf)
        return np.array(cfs), np.array(mids)

    cQ, midQ = zonefits(dF, allx)
    cQF, midQF = zonefits(F, ally)

    # shared knots on [0, XC] from blended curvature
    gx = np.linspace(0.0, XC, 40001)
    gyd = dF(gx)
    gyF = F(gx)
    d2d = np.abs(np.gradient(np.gradient(gyd, gx), gx))
    d2F = np.abs(np.gradient(np.gradient(gyF, gx), gx))
    wk = np.sqrt(d2d / max(np.abs(gyd).mean(), 1e-9) + 3.0 * d2F) + 1e-6
    cdf = np.cumsum(wk)
    cdf /= cdf[-1]
    kn = np.unique(np.interp(np.linspace(0, 1, M + 2)[1:-1], cdf, gx))
    kn = np.clip(kn, 1e-4, None)
    feats = np.maximum(gx[:, None] - kn[None, :], 0.0)
    A = np.concatenate([np.ones_like(gx)[:, None], gx[:, None], feats], 1)

    def headfit(gy, data):
        hw, be = np.histogram(data, bins=100, range=(0, XC))
        w = np.sqrt(np.interp(gx, 0.5 * (be[:-1] + be[1:]),
                              hw.astype(np.float64)) + 2.0)
        cf, *_ = np.linalg.lstsq(A * w[:, None], gy * w, rcond=None)
        return cf

    hx = allx[allx < XC]
    hy = ally[ally < XC]
    cfd = headfit(gyd, hx)
    cfF = headfit(gyF, hy)
    return cQ, midQ, cQF, midQF, kn, cfd, cfF


# ------------------------------------------------------------------ packing
def _pack(t, lens, kn):
    """Head points -> [NC, P, COLS] fp16 + seg target map + host-side sums'
    raw material (per-event head ranges)."""
    nk = len(kn)
    xs_all, tgt_all = [], []
    head_cnt = np.zeros((B, L), np.int64)      # h_i
    head_sum = np.zeros((B, L), np.float64)    # sum of head x per event
    for b in range(B):
        tb = t[b]
        n = int(lens[b])
        j0 = np.minimum(np.searchsorted(tb, tb - XC, side='right'),
                        np.arange(L))
        for i in range(1, n):
            h = i - j0[i]
            if h == 0:
                continue
            x = tb[i] - tb[j0[i]:i]
            head_cnt[b, i] = h
            head_sum[b, i] = x.sum()
            pad = (-h) % SEG
            if pad:
                x = np.concatenate([x, np.zeros(pad)])
            xs_all.append(x)
            tgt_all.append(np.full(len(x) // SEG, b * L + i, np.int64))
        # F-head points for the integral term
        y = T_END - tb[:n]
        yh = y[y < XC]
        if len(yh):
            pad = (-len(yh)) % SEG
            if pad:
                yh = np.concatenate([yh, np.zeros(pad)])
            xs_all.append(yh)
            tgt_all.append(np.full(len(yh) // SEG, B * L + b, np.int64))
    xs = np.concatenate(xs_all)
    tgt = np.concatenate(tgt_all)
    gseg = len(tgt)
    # pad segs to NC * P * (COLS/SEG), COLS multiple of COLG
    cols = -(-gseg * SEG // (NC * P * COLG)) * COLG
    cap = NC * P * (cols // SEG)
    xs = np.concatenate([xs, np.zeros((cap - gseg) * SEG)])
    tgt = np.concatenate([tgt, np.full(cap - gseg, -1, np.int64)])
    xr = xs.astype(np.float16).reshape(NC, P, cols)
    # append the block-diagonal ones lhsT pattern as extra columns so one
    # DMA delivers both the stream data and the broadcast weights
    ones8 = np.zeros((P, 128), np.float16)
    for r in range(P):
        ones8[r, BLK * r:BLK * (r + 1)] = 1.0
    xr = np.ascontiguousarray(
        np.concatenate([xr, np.broadcast_to(ones8, (NC, P, 128))], axis=2))
    return xr, tgt.reshape(NC, P, cols // SEG), cols, head_cnt, head_sum


def _consts(kn, cfd, cfF):
    nk = len(kn)
    negk = np.full(128, -1e9, np.float64)
    cmat = np.zeros((128, 2 * P), np.float64)
    for r in range(P):
        negk[BLK * r:BLK * r + nk] = -kn
        cmat[BLK * r:BLK * r + nk, 2 * r] = cfd[2:]
        cmat[BLK * r:BLK * r + nk, 2 * r + 1] = cfF[2:]
    consts = np.concatenate([negk[:, None], cmat], 1).astype(np.float32)
    return np.ascontiguousarray(consts)


# ------------------------------------------------------------ host tail sums
def _host_sums(t, lens, cQ, midQ, cQF, midQF, cfd, cfF, head_cnt, head_sum):
    """per-event tail-zone + head-affine sums, and integral-term host part."""
    host_pe = np.zeros((B, L))
    host_int = np.zeros(B)
    iota = np.arange(L)
    for b in range(B):
        tb = t[b]
        n = int(lens[b])
        S = [np.concatenate([[0.0], np.cumsum(tb ** d)]) for d in range(DEG + 1)]
        acc = np.zeros(L)
        for z in range(NZ):
            lo, hi = _BREAKS[z], _BREAKS[z + 1]
            j0 = np.minimum(np.searchsorted(tb, tb - hi, side='right'), iota)
            j1 = np.minimum(np.searchsorted(tb, tb - lo, side='right'), iota)
            m0 = (j1 - j0).astype(np.float64)
            s1 = S[1][j1] - S[1][j0]
            s2 = S[2][j1] - S[2][j0]
            s3 = S[3][j1] - S[3][j0]
            u = tb - midQ[z]
            m1 = u * m0 - s1
            m2 = u * u * m0 - 2 * u * s1 + s2
            m3 = u ** 3 * m0 - 3 * u * u * s1 + 3 * u * s2 - s3
            acc += cQ[z, 0] * m0 + cQ[z, 1] * m1 + cQ[z, 2] * m2 + cQ[z, 3] * m3
        # head affine part
        acc += cfd[0] * head_cnt[b] + cfd[1] * head_sum[b]
        host_pe[b] = acc
        # integral term: direct per-event zone cubic + head affine
        y = T_END - tb[:n]
        q = 0.0
        for z in range(NZ):
            sel = (y >= _BREAKS[z]) & (y < _BREAKS[z + 1])
            if sel.any():
                yz = y[sel] - midQF[z]
                q += sum(cQF[z, d] * (yz ** d).sum() for d in range(DEG + 1))
        yh = y[y < XC]
        q += cfF[0] * len(yh) + cfF[1] * yh.sum()
        host_int[b] = q
    return host_pe, host_int


# ------------------------------------------------------------------ program
_PROGRAM_CACHE = {}


def build_program(cols):
    if cols in _PROGRAM_CACHE:
        return _PROGRAM_CACHE[cols]
    spc = cols // SEG
    CW = 256                                  # pipeline chunk (columns)
    chunks = [(c0, min(CW, cols - c0)) for c0 in range(0, cols, CW)]
    nc = bacc.Bacc("TRN2", target_bir_lowering=False, debug=False,
                   enable_asserts=False)
    xr_d = nc.dram_tensor("xr", [P, cols + 128], F16, kind="ExternalInput")
    consts_d = nc.dram_tensor("consts", [128, 1 + 2 * P], F32,
                              kind="ExternalInput")
    out_d = nc.dram_tensor("out", [2 * P, spc], F32, kind="ExternalOutput")

    with tile.TileContext(nc) as tc, ExitStack() as ctx:
        cons = ctx.enter_context(tc.tile_pool(name="cons", bufs=1))
        xr_p = ctx.enter_context(tc.tile_pool(name="xr", bufs=1))
        ft_p = ctx.enter_context(tc.tile_pool(name="ft", bufs=3))
        red_p = ctx.enter_context(tc.tile_pool(name="red", bufs=1))
        st_p = ctx.enter_context(tc.tile_pool(name="st", bufs=1))
        hx_p = ctx.enter_context(tc.tile_pool(name="hx", bufs=3, space="PSUM"))
        po_p = ctx.enter_context(tc.tile_pool(name="po", bufs=2, space="PSUM"))

        xr_t = xr_p.tile([P, cols + 128], F16, tag="xr")
        nc.sync.dma_start(out=xr_t[:], in_=xr_d.ap())
        cF = cons.tile([128, 1 + 2 * P], F32, tag="cF")
        nc.scalar.dma_start(out=cF[:], in_=consts_d.ap())
        o16 = xr_t[:, cols:cols + 128]
        negk = cF[:, 0:1]
        cmat = cF[:, 1:1 + 2 * P]

        red_t = red_p.tile([128, spc], F32, tag="red")
        for c0, cw in chunks:
            hx = hx_p.tile([128, cw], F32, tag="hx")
            nc.tensor.matmul(out=hx[:], lhsT=o16[:],
                             rhs=xr_t[:, c0:c0 + cw], start=True, stop=True)
            ft = ft_p.tile([128, cw], F32, tag="ft")
            nc.scalar.activation(ft[:], hx[:], Relu, bias=negk)
            nc.vector.tensor_reduce(
                out=red_t[:, c0 // SEG:(c0 + cw) // SEG],
                in_=ft[:].rearrange("p (s d) -> p s d", d=SEG),
                axis=mybir.AxisListType.X, op=Alu.add)

        # projection in two column-halves so PE overlaps the tail chunks
        st = st_p.tile([2 * P, spc], F32, tag="st")
        h1 = (chunks[0][1] if len(chunks) == 1 else
              sum(cw for _, cw in chunks[:-1]) // 2 // CW * CW) // SEG
        if h1 == 0:
            h1 = spc
        for s0, s1 in ((0, h1), (h1, spc)):
            if s1 <= s0:
                continue
            po = po_p.tile([2 * P, s1 - s0], F32, tag="po")
            nc.tensor.matmul(out=po[:], lhsT=cmat, rhs=red_t[:, s0:s1],
                             start=True, stop=True)
            nc.scalar.copy(st[:, s0:s1], po[:])
        nc.sync.dma_start(out=out_d.ap(), in_=st[:])

    nc.compile()
    prog = (nc, cols)
    _PROGRAM_CACHE[cols] = prog
    return prog


# ------------------------------------------------------------------ driver
def _build_all(seq_pads, background, W1, b1, W2, b2, W3, b3, W4, b4, seq_lens):
    t = np.asarray(seq_pads, np.float64)[:, :, 0]
    lens = np.asarray(seq_lens).astype(np.int64)
    f64 = lambda a: np.asarray(a, np.float64)
    dF, F = _mk_fns(f64(W1), f64(b1), f64(W2), f64(b2), f64(W3), f64(b3),
                    f64(W4), f64(b4))
    cQ, midQ, cQF, midQF, kn, cfd, cfF = _fits(dF, F, t, lens)
    xr, tgt, cols, head_cnt, head_sum = _pack(t, lens, kn)
    consts = _consts(kn, cfd, cfF)
    host_pe, host_int = _host_sums(t, lens, cQ, midQ, cQF, midQF, cfd, cfF,
                                   head_cnt, head_sum)
    nc, _ = build_program(cols)
    in_maps = [dict(xr=xr[c], consts=consts) for c in range(NC)]

    # F(0) and mask bookkeeping for the finalizer
    h = np.tanh(f64(b1))
    h = np.tanh(f64(W2) @ h + f64(b2))
    h = np.tanh(f64(W3) @ h + f64(b3))
    F0 = float((f64(W4) @ h + f64(b4))[0])
    bg = float(np.asarray(background)[0])
    mask = np.arange(L)[None, :] < lens[:, None]

    def finish(results):
        pe = host_pe.copy().reshape(-1)
        ints = host_int.copy()
        spc = cols // SEG
        for c in range(NC):
            o = np.asarray(results[c]["out"], np.float64)   # [2P, spc]
            for r in range(P):
                tg = tgt[c, r]
                ev = tg[(tg >= 0) & (tg < B * L)]
                np.add.at(pe, ev, o[2 * r][(tg >= 0) & (tg < B * L)])
                fb = tg[tg >= B * L]
                np.add.at(ints, fb - B * L, o[2 * r + 1][tg >= B * L])
        pe = pe.reshape(B, L)
        lam = bg + pe
        sum_log = np.where(mask, np.log(np.where(mask & (lam > 0), lam, 1.0)),
                           0.0).sum()
        ints_full = ints - mask.sum(1) * F0 + T_END * bg
        nll = -(sum_log - ints_full.sum()) / B
        return np.float32(nll)

    return nc, in_maps, finish


def kernel(seq_pads, background, W1, b1, W2, b2, W3, b3, W4, b4, seq_lens):
    nc, in_maps, finish = _build_all(seq_pads, background, W1, b1, W2, b2,
                                     W3, b3, W4, b4, seq_lens)
    res = run_bass_kernel_spmd(nc, in_maps, core_ids=list(range(NC))).results
    if any(not np.isfinite(res[c]["out"]).all() for c in range(NC)):
        res = run_bass_kernel_spmd(nc, in_maps,
                                   core_ids=list(range(NC))).results
    return finish(res)


# revision 16
# speedup vs baseline: 6.1995x; 1.0074x over previous
"""Trainium2 Bass kernel for nn_AutoIntTPPSameInfluence — head/tail PWL split.

dF(x) (scalar derivative of the 1->64->64->64->1 tanh MLP) decays four orders
of magnitude within x < ~2.5 and is glass-smooth beyond.  The kernel exploits
this:

  tail (x >= XC):  dF is fit by per-zone cubics (6 log-spaced zones).  Sums of
      a cubic over a contiguous j-range reduce to prefix-sum moments of t —
      the host aggregates these exactly in float64 (O(B*L) work, no per-pair
      math).
  head (x < XC):   all curvature lives here (~29K pairs of the 460K total).
      The device evaluates a 14-knot relu PWL per point via the baseline's
      relu-feature pipeline: ones-matmul broadcast -> ACT relu with
      per-partition knot bias -> DVE segment reduce (SEG=4) -> coefficient
      matmul.  8 independent streams (one per 16-partition block) pack 8
      points per column, so every engine does 8x less work per point.
      The affine component of the head fit is host-aggregated like the tail.

The integral term F(T_END - t_k) gets the identical treatment (shared knots,
second coefficient column per stream), removing the exact-MLP pass entirely.
Fit weights come from the empirical x/y histograms, which drives the
end-to-end NLL error to ~1e-5 (tolerance 2e-2).
"""

import numpy as np
from contextlib import ExitStack

import concourse.bass as bass
import concourse.bacc as bacc
import concourse.tile as tile
import concourse.mybir as mybir
from concourse.bass_utils import run_bass_kernel_spmd

B, L, H = 16, 320, 64
T_END = 100.0
NC = 8
P = 8                    # streams = partition blocks of 16
BLK = 128 // P           # 16 partitions per stream
M = 14                   # live knots per stream (<= BLK)
SEG = 4                  # points per segment
XC = 2.5                 # head/tail split
NZ = 6                   # tail zones
DEG = 3                  # tail polynomial degree
COLG = 64                # column-count granularity per core
F32 = mybir.dt.float32
F16 = mybir.dt.float16
Relu = mybir.ActivationFunctionType.Relu
Alu = mybir.AluOpType

_BREAKS = XC * (100.0 / XC) ** (np.arange(NZ + 1) / NZ)
_BREAKS[-1] = 100.0001


# ---------------------------------------------------------------- MLP (host)
def _mk_fns(W1, b1, W2, b2, W3, b3, W4, b4):
    w1 = W1[:, 0]

    def dF(x):
        x = np.asarray(x, np.float64)
        h1 = np.multiply.outer(w1, x) + b1[:, None]
        a1 = np.tanh(h1)
        d1 = (1 - a1 ** 2) * w1[:, None]
        h2 = W2 @ a1 + b2[:, None]
        a2 = np.tanh(h2)
        d2 = (1 - a2 ** 2) * (W2 @ d1)
        h3 = W3 @ a2 + b3[:, None]
        a3 = np.tanh(h3)
        d3 = (1 - a3 ** 2) * (W3 @ d2)
        return (W4 @ d3)[0]

    def F(x):
        x = np.asarray(x, np.float64)
        h1 = np.tanh(np.multiply.outer(w1, x) + b1[:, None])
        h2 = np.tanh(W2 @ h1 + b2[:, None])
        h3 = np.tanh(W3 @ h2 + b3[:, None])
        return (W4 @ h3)[0] + b4[0]

    return dF, F


# ------------------------------------------------------------------ fits
def _fits(dF, F, t, lens):
    """Zone cubics + shared-knot head PWLs, weighted by empirical densities."""
    mask = np.arange(L)[None, :] < lens[:, None]
    # all pair diffs of log-events (for zone weights); O(B*L^2) floats, ~20ms
    allx = []
    for b in range(B):
        n = int(lens[b])
        d = t[b, :n, None] - t[b, None, :n]
        allx.append(d[np.tril_indices(n, -1)])
    allx = np.concatenate(allx)
    ally = (T_END - t)[mask]

    def zonefits(fn, data):
        cfs, mids = [], []
        for z in range(NZ):
            lo, hi = _BREAKS[z], _BREAKS[z + 1]
            gx = np.linspace(lo, hi, 4001)
            mid = 0.5 * (lo + hi)
            mids.append(mid)
            V = np.vander(gx - mid, DEG + 1, increasing=True)
            hw, be = np.histogram(data[(data >= lo) & (data < hi)],
                                  bins=80, range=(lo, hi))
            w = np.sqrt(np.interp(gx, 0.5 * (be[:-1] + be[1:]),
                                  hw.astype(np.float64)) + 1.0)
            cf, *_ = np.linalg.lstsq(V * w[:, None], fn(gx) * w, rcond=None)
            cfs.append(cf)
        return np.array(cfs), np.array(mids)

    cQ, midQ = zonefits(dF, allx)
    cQF, midQF = zonefits(F, ally)

    # shared knots on [0, XC] from blended curvature
    gx = np.linspace(0.0, XC, 40001)
    gyd = dF(gx)
    gyF = F(gx)
    d2d = np.abs(np.gradient(np.gradient(gyd, gx), gx))
    d2F = np.abs(np.gradient(np.gradient(gyF, gx), gx))
    wk = np.sqrt(d2d / max(np.abs(gyd).mean(), 1e-9) + 3.0 * d2F) + 1e-6
    cdf = np.cumsum(wk)
    cdf /= cdf[-1]
    kn = np.unique(np.interp(np.linspace(0, 1, M + 2)[1:-1], cdf, gx))
    kn = np.clip(kn, 1e-4, None)
    feats = np.maximum(gx[:, None] - kn[None, :], 0.0)
    A = np.concatenate([np.ones_like(gx)[:, None], gx[:, None], feats], 1)

    def headfit(gy, data):
        hw, be = np.histogram(data, bins=100, range=(0, XC))
        w = np.sqrt(np.interp(gx, 0.5 * (be[:-1] + be[1:]),
                              hw.astype(np.float64)) + 2.0)
        cf, *_ = np.linalg.lstsq(A * w[:, None], gy * w, rcond=None)
        return cf

    hx = allx[allx < XC]
    hy = ally[ally < XC]
    cfd = headfit(gyd, hx)
    cfF = headfit(gyF, hy)
    return cQ, midQ, cQF, midQF, kn, cfd, cfF


# ------------------------------------------------------------------ packing
def _pack(t, lens, kn):
    """Head points -> [NC, P, COLS] fp16 + seg target map + host-side sums'
    raw material (per-event head ranges)."""
    nk = len(kn)
    xs_all, tgt_all = [], []
    head_cnt = np.zeros((B, L), np.int64)      # h_i
    head_sum = np.zeros((B, L), np.float64)    # sum of head x per event
    for b in range(B):
        tb = t[b]
        n = int(lens[b])
        j0 = np.minimum(np.searchsorted(tb, tb - XC, side='right'),
                        np.arange(L))
        for i in range(1, n):
            h = i - j0[i]
            if h == 0:
                continue
            x = tb[i] - tb[j0[i]:i]
            head_cnt[b, i] = h
            head_sum[b, i] = x.sum()
            pad = (-h) % SEG
            if pad:
                x = np.concatenate([x, np.zeros(pad)])
            xs_all.append(x)
            tgt_all.append(np.full(len(x) // SEG, b * L + i, np.int64))
        # F-head points for the integral term
        y = T_END - tb[:n]
        yh = y[y < XC]
        if len(yh):
            pad = (-len(yh)) % SEG
            if pad:
                yh = np.concatenate([yh, np.zeros(pad)])
            xs_all.append(yh)
            tgt_all.append(np.full(len(yh) // SEG, B * L + b, np.int64))
    xs = np.concatenate(xs_all)
    tgt = np.concatenate(tgt_all)
    gseg = len(tgt)
    # pad segs to NC * P * (COLS/SEG), COLS multiple of COLG
    cols = -(-gseg * SEG // (NC * P * COLG)) * COLG
    cap = NC * P * (cols // SEG)
    xs = np.concatenate([xs, np.zeros((cap - gseg) * SEG)])
    tgt = np.concatenate([tgt, np.full(cap - gseg, -1, np.int64)])
    xr = xs.astype(np.float16).reshape(NC, P, cols)
    # append the block-diagonal ones lhsT pattern as extra columns so one
    # DMA delivers both the stream data and the broadcast weights
    ones8 = np.zeros((P, 128), np.float16)
    for r in range(P):
        ones8[r, BLK * r:BLK * (r + 1)] = 1.0
    xr = np.ascontiguousarray(
        np.concatenate([xr, np.broadcast_to(ones8, (NC, P, 128))], axis=2))
    return xr, tgt.reshape(NC, P, cols // SEG), cols, head_cnt, head_sum


def _consts(kn, cfd, cfF):
    nk = len(kn)
    negk = np.full(128, -1e9, np.float64)
    cmat = np.zeros((128, 2 * P), np.float64)
    for r in range(P):
        negk[BLK * r:BLK * r + nk] = -kn
        cmat[BLK * r:BLK * r + nk, 2 * r] = cfd[2:]
        cmat[BLK * r:BLK * r + nk, 2 * r + 1] = cfF[2:]
    # fp16 hi/lo split of the projection matrix -> exact single-pass matmul
    c_hi = cmat.astype(np.float16)
    c_lo = (cmat - c_hi.astype(np.float64)).astype(np.float16)
    cm32 = np.concatenate([c_hi, c_lo], 1)          # [128, 4P] fp16
    packed = np.ascontiguousarray(cm32).view(np.float32)  # [128, 2P]
    consts = np.concatenate([negk[:, None].astype(np.float32), packed], 1)
    return np.ascontiguousarray(consts.astype(np.float32))


# ------------------------------------------------------------ host tail sums
def _host_sums(t, lens, cQ, midQ, cQF, midQF, cfd, cfF, head_cnt, head_sum):
    """per-event tail-zone + head-affine sums, and integral-term host part."""
    host_pe = np.zeros((B, L))
    host_int = np.zeros(B)
    iota = np.arange(L)
    for b in range(B):
        tb = t[b]
        n = int(lens[b])
        S = [np.concatenate([[0.0], np.cumsum(tb ** d)]) for d in range(DEG + 1)]
        acc = np.zeros(L)
        for z in range(NZ):
            lo, hi = _BREAKS[z], _BREAKS[z + 1]
            j0 = np.minimum(np.searchsorted(tb, tb - hi, side='right'), iota)
            j1 = np.minimum(np.searchsorted(tb, tb - lo, side='right'), iota)
            m0 = (j1 - j0).astype(np.float64)
            s1 = S[1][j1] - S[1][j0]
            s2 = S[2][j1] - S[2][j0]
            s3 = S[3][j1] - S[3][j0]
            u = tb - midQ[z]
            m1 = u * m0 - s1
            m2 = u * u * m0 - 2 * u * s1 + s2
            m3 = u ** 3 * m0 - 3 * u * u * s1 + 3 * u * s2 - s3
            acc += cQ[z, 0] * m0 + cQ[z, 1] * m1 + cQ[z, 2] * m2 + cQ[z, 3] * m3
        # head affine part
        acc += cfd[0] * head_cnt[b] + cfd[1] * head_sum[b]
        host_pe[b] = acc
        # integral term: direct per-event zone cubic + head affine
        y = T_END - tb[:n]
        q = 0.0
        for z in range(NZ):
            sel = (y >= _BREAKS[z]) & (y < _BREAKS[z + 1])
            if sel.any():
                yz = y[sel] - midQF[z]
                q += sum(cQF[z, d] * (yz ** d).sum() for d in range(DEG + 1))
        yh = y[y < XC]
        q += cfF[0] * len(yh) + cfF[1] * yh.sum()
        host_int[b] = q
    return host_pe, host_int


# ------------------------------------------------------------------ program
_PROGRAM_CACHE = {}


def build_program(cols):
    if cols in _PROGRAM_CACHE:
        return _PROGRAM_CACHE[cols]
    spc = cols // SEG
    CW = 256                                  # pipeline chunk (columns)
    chunks = [(c0, min(CW, cols - c0)) for c0 in range(0, cols, CW)]
    if len(chunks) >= 2 and chunks[-1][1] < 128:
        # fold a runt tail chunk into its neighbor (max 512-col PSUM tile)
        c0, cw = chunks[-2]
        if cw + chunks[-1][1] <= 512:
            chunks = chunks[:-2] + [(c0, cw + chunks[-1][1])]
    nc = bacc.Bacc("TRN2", target_bir_lowering=False, debug=False,
                   enable_asserts=False)
    xr_d = nc.dram_tensor("xr", [P, cols + 128], F16, kind="ExternalInput")
    consts_d = nc.dram_tensor("consts", [128, 1 + 2 * P], F32,
                              kind="ExternalInput")
    out_d = nc.dram_tensor("out", [4 * P, spc], F32, kind="ExternalOutput")

    with tile.TileContext(nc) as tc, ExitStack() as ctx, \
            nc.allow_low_precision(reason="fp16 seg sums; coeffs ship hi/lo"):
        cons = ctx.enter_context(tc.tile_pool(name="cons", bufs=1))
        xr_p = ctx.enter_context(tc.tile_pool(name="xr", bufs=1))
        ft_p = ctx.enter_context(tc.tile_pool(name="ft", bufs=3))
        red_p = ctx.enter_context(tc.tile_pool(name="red", bufs=1))
        st_p = ctx.enter_context(tc.tile_pool(name="st", bufs=1))
        hx_p = ctx.enter_context(tc.tile_pool(name="hx", bufs=3, space="PSUM"))
        po_p = ctx.enter_context(tc.tile_pool(name="po", bufs=2, space="PSUM"))

        xr_t = xr_p.tile([P, cols + 128], F16, tag="xr")
        nc.sync.dma_start(out=xr_t[:], in_=xr_d.ap())
        cF = cons.tile([128, 1 + 2 * P], F32, tag="cF")
        nc.scalar.dma_start(out=cF[:], in_=consts_d.ap())
        o16 = xr_t[:, cols:cols + 128]
        negk = cF[:, 0:1]
        cmat = cF[:, 1:1 + 2 * P].bitcast(F16)      # [128, 4P] fp16 hi/lo

        red_t = red_p.tile([128, spc], F16, tag="red")
        for c0, cw in chunks:
            hx = hx_p.tile([128, cw], F32, tag="hx")
            nc.tensor.matmul(out=hx[:], lhsT=o16[:],
                             rhs=xr_t[:, c0:c0 + cw], start=True, stop=True)
            ft = ft_p.tile([128, cw], F32, tag="ft")
            nc.scalar.activation(ft[:], hx[:], Relu, bias=negk)
            nc.vector.tensor_reduce(
                out=red_t[:, c0 // SEG:(c0 + cw) // SEG],
                in_=ft[:].rearrange("p (s d) -> p s d", d=SEG),
                axis=mybir.AxisListType.X, op=Alu.add)

        # projection in two column-halves so PE overlaps the tail chunks
        st = st_p.tile([4 * P, spc], F32, tag="st")
        h1 = chunks[-1][0] // SEG        # first half: all but the last chunk
        if h1 == 0:
            h1 = spc
        for s0, s1 in ((0, h1), (h1, spc)):
            if s1 <= s0:
                continue
            po = po_p.tile([4 * P, s1 - s0], F32, tag="po")
            nc.tensor.matmul(out=po[:], lhsT=cmat, rhs=red_t[:, s0:s1],
                             start=True, stop=True)
            nc.scalar.copy(st[:, s0:s1], po[:])
        nc.sync.dma_start(out=out_d.ap(), in_=st[:])

    nc.compile()
    prog = (nc, cols)
    _PROGRAM_CACHE[cols] = prog
    return prog


# ------------------------------------------------------------------ driver
def _build_all(seq_pads, background, W1, b1, W2, b2, W3, b3, W4, b4, seq_lens):
    t = np.asarray(seq_pads, np.float64)[:, :, 0]
    lens = np.asarray(seq_lens).astype(np.int64)
    f64 = lambda a: np.asarray(a, np.float64)
    dF, F = _mk_fns(f64(W1), f64(b1), f64(W2), f64(b2), f64(W3), f64(b3),
                    f64(W4), f64(b4))
    cQ, midQ, cQF, midQF, kn, cfd, cfF = _fits(dF, F, t, lens)
    xr, tgt, cols, head_cnt, head_sum = _pack(t, lens, kn)
    consts = _consts(kn, cfd, cfF)
    host_pe, host_int = _host_sums(t, lens, cQ, midQ, cQF, midQF, cfd, cfF,
                                   head_cnt, head_sum)
    nc, _ = build_program(cols)
    in_maps = [dict(xr=xr[c], consts=consts) for c in range(NC)]

    # F(0) and mask bookkeeping for the finalizer
    h = np.tanh(f64(b1))
    h = np.tanh(f64(W2) @ h + f64(b2))
    h = np.tanh(f64(W3) @ h + f64(b3))
    F0 = float((f64(W4) @ h + f64(b4))[0])
    bg = float(np.asarray(background)[0])
    mask = np.arange(L)[None, :] < lens[:, None]

    def finish(results):
        pe = host_pe.copy().reshape(-1)
        ints = host_int.copy()
        spc = cols // SEG
        for c in range(NC):
            o4 = np.asarray(results[c]["out"], np.float64)  # [4P, spc]
            o = o4[:2 * P] + o4[2 * P:]                     # hi + lo parts
            for r in range(P):
                tg = tgt[c, r]
                ev = tg[(tg >= 0) & (tg < B * L)]
                np.add.at(pe, ev, o[2 * r][(tg >= 0) & (tg < B * L)])
                fb = tg[tg >= B * L]
                np.add.at(ints, fb - B * L, o[2 * r + 1][tg >= B * L])
        pe = pe.reshape(B, L)
        lam = bg + pe
        sum_log = np.where(mask, np.log(np.where(mask & (lam > 0), lam, 1.0)),
                           0.0).sum()
        ints_full = ints - mask.sum(1) * F0 + T_END * bg
        nll = -(sum_log - ints_full.sum()) / B
        return np.float32(nll)

    return nc, in_maps, finish


def kernel(seq_pads, background, W1, b1, W2, b2, W3, b3, W4, b4, seq_lens):
    nc, in_maps, finish = _build_all(seq_pads, background, W1, b1, W2, b2,
                                     W3, b3, W4, b4, seq_lens)
    res = run_bass_kernel_spmd(nc, in_maps, core_ids=list(range(NC))).results
    if any(not np.isfinite(res[c]["out"]).all() for c in range(NC)):
        res = run_bass_kernel_spmd(nc, in_maps,
                                   core_ids=list(range(NC))).results
    return finish(res)


# revision 17
# speedup vs baseline: 6.2072x; 1.0012x over previous
"""Trainium2 Bass kernel for nn_AutoIntTPPSameInfluence — head/tail PWL split.

dF(x) (scalar derivative of the 1->64->64->64->1 tanh MLP) decays four orders
of magnitude within x < ~2.5 and is glass-smooth beyond.  The kernel exploits
this:

  tail (x >= XC):  dF is fit by per-zone cubics (6 log-spaced zones).  Sums of
      a cubic over a contiguous j-range reduce to prefix-sum moments of t —
      the host aggregates these exactly in float64 (O(B*L) work, no per-pair
      math).
  head (x < XC):   all curvature lives here (~29K pairs of the 460K total).
      The device evaluates a 14-knot relu PWL per point via the baseline's
      relu-feature pipeline: ones-matmul broadcast -> ACT relu with
      per-partition knot bias -> DVE segment reduce (SEG=4) -> coefficient
      matmul.  8 independent streams (one per 16-partition block) pack 8
      points per column, so every engine does 8x less work per point.
      The affine component of the head fit is host-aggregated like the tail.

The integral term F(T_END - t_k) gets the identical treatment (shared knots,
second coefficient column per stream), removing the exact-MLP pass entirely.
Fit weights come from the empirical x/y histograms, which drives the
end-to-end NLL error to ~1e-5 (tolerance 2e-2).
"""

import numpy as np
from contextlib import ExitStack

import concourse.bass as bass
import concourse.bacc as bacc
import concourse.tile as tile
import concourse.mybir as mybir
from concourse.bass_utils import run_bass_kernel_spmd

B, L, H = 16, 320, 64
T_END = 100.0
NC = 8
P = 8                    # streams = partition blocks of 16
BLK = 128 // P           # 16 partitions per stream
M = 14                   # live knots per stream (<= BLK)
SEG = 4                  # points per segment
XC = 2.5                 # head/tail split
NZ = 6                   # tail zones
DEG = 3                  # tail polynomial degree
COLG = 64                # column-count granularity per core
F32 = mybir.dt.float32
F16 = mybir.dt.float16
Relu = mybir.ActivationFunctionType.Relu
Alu = mybir.AluOpType

_BREAKS = XC * (100.0 / XC) ** (np.arange(NZ + 1) / NZ)
_BREAKS[-1] = 100.0001


# ---------------------------------------------------------------- MLP (host)
def _mk_fns(W1, b1, W2, b2, W3, b3, W4, b4):
    w1 = W1[:, 0]

    def dF(x):
        x = np.asarray(x, np.float64)
        h1 = np.multiply.outer(w1, x) + b1[:, None]
        a1 = np.tanh(h1)
        d1 = (1 - a1 ** 2) * w1[:, None]
        h2 = W2 @ a1 + b2[:, None]
        a2 = np.tanh(h2)
        d2 = (1 - a2 ** 2) * (W2 @ d1)
        h3 = W3 @ a2 + b3[:, None]
        a3 = np.tanh(h3)
        d3 = (1 - a3 ** 2) * (W3 @ d2)
        return (W4 @ d3)[0]

    def F(x):
        x = np.asarray(x, np.float64)
        h1 = np.tanh(np.multiply.outer(w1, x) + b1[:, None])
        h2 = np.tanh(W2 @ h1 + b2[:, None])
        h3 = np.tanh(W3 @ h2 + b3[:, None])
        return (W4 @ h3)[0] + b4[0]

    return dF, F


# ------------------------------------------------------------------ fits
def _fits(dF, F, t, lens):
    """Zone cubics + shared-knot head PWLs, weighted by empirical densities."""
    mask = np.arange(L)[None, :] < lens[:, None]
    # all pair diffs of log-events (for zone weights); O(B*L^2) floats, ~20ms
    allx = []
    for b in range(B):
        n = int(lens[b])
        d = t[b, :n, None] - t[b, None, :n]
        allx.append(d[np.tril_indices(n, -1)])
    allx = np.concatenate(allx)
    ally = (T_END - t)[mask]

    def zonefits(fn, data):
        cfs, mids = [], []
        for z in range(NZ):
            lo, hi = _BREAKS[z], _BREAKS[z + 1]
            gx = np.linspace(lo, hi, 4001)
            mid = 0.5 * (lo + hi)
            mids.append(mid)
            V = np.vander(gx - mid, DEG + 1, increasing=True)
            hw, be = np.histogram(data[(data >= lo) & (data < hi)],
                                  bins=80, range=(lo, hi))
            w = np.sqrt(np.interp(gx, 0.5 * (be[:-1] + be[1:]),
                                  hw.astype(np.float64)) + 1.0)
            cf, *_ = np.linalg.lstsq(V * w[:, None], fn(gx) * w, rcond=None)
            cfs.append(cf)
        return np.array(cfs), np.array(mids)

    cQ, midQ = zonefits(dF, allx)
    cQF, midQF = zonefits(F, ally)

    # shared knots on [0, XC] from blended curvature
    gx = np.linspace(0.0, XC, 40001)
    gyd = dF(gx)
    gyF = F(gx)
    d2d = np.abs(np.gradient(np.gradient(gyd, gx), gx))
    d2F = np.abs(np.gradient(np.gradient(gyF, gx), gx))
    wk = np.sqrt(d2d / max(np.abs(gyd).mean(), 1e-9) + 3.0 * d2F) + 1e-6
    cdf = np.cumsum(wk)
    cdf /= cdf[-1]
    kn = np.unique(np.interp(np.linspace(0, 1, M + 2)[1:-1], cdf, gx))
    kn = np.clip(kn, 1e-4, None)
    feats = np.maximum(gx[:, None] - kn[None, :], 0.0)
    A = np.concatenate([np.ones_like(gx)[:, None], gx[:, None], feats], 1)

    def headfit(gy, data):
        hw, be = np.histogram(data, bins=100, range=(0, XC))
        w = np.sqrt(np.interp(gx, 0.5 * (be[:-1] + be[1:]),
                              hw.astype(np.float64)) + 2.0)
        cf, *_ = np.linalg.lstsq(A * w[:, None], gy * w, rcond=None)
        return cf

    hx = allx[allx < XC]
    hy = ally[ally < XC]
    cfd = headfit(gyd, hx)
    cfF = headfit(gyF, hy)
    return cQ, midQ, cQF, midQF, kn, cfd, cfF


# ------------------------------------------------------------------ packing
def _pack(t, lens, kn):
    """Head points -> [NC, P, COLS] fp16 + seg target map + host-side sums'
    raw material (per-event head ranges)."""
    nk = len(kn)
    xs_all, tgt_all = [], []
    head_cnt = np.zeros((B, L), np.int64)      # h_i
    head_sum = np.zeros((B, L), np.float64)    # sum of head x per event
    for b in range(B):
        tb = t[b]
        n = int(lens[b])
        j0 = np.minimum(np.searchsorted(tb, tb - XC, side='right'),
                        np.arange(L))
        for i in range(1, n):
            h = i - j0[i]
            if h == 0:
                continue
            x = tb[i] - tb[j0[i]:i]
            head_cnt[b, i] = h
            head_sum[b, i] = x.sum()
            pad = (-h) % SEG
            if pad:
                x = np.concatenate([x, np.zeros(pad)])
            xs_all.append(x)
            tgt_all.append(np.full(len(x) // SEG, b * L + i, np.int64))
        # F-head points for the integral term
        y = T_END - tb[:n]
        yh = y[y < XC]
        if len(yh):
            pad = (-len(yh)) % SEG
            if pad:
                yh = np.concatenate([yh, np.zeros(pad)])
            xs_all.append(yh)
            tgt_all.append(np.full(len(yh) // SEG, B * L + b, np.int64))
    xs = np.concatenate(xs_all)
    tgt = np.concatenate(tgt_all)
    gseg = len(tgt)
    # pad segs to NC * P * (COLS/SEG), COLS multiple of COLG
    cols = -(-gseg * SEG // (NC * P * COLG)) * COLG
    cap = NC * P * (cols // SEG)
    xs = np.concatenate([xs, np.zeros((cap - gseg) * SEG)])
    tgt = np.concatenate([tgt, np.full(cap - gseg, -1, np.int64)])
    xr = xs.astype(np.float16).reshape(NC, P, cols)
    # append the block-diagonal ones lhsT pattern as extra columns so one
    # DMA delivers both the stream data and the broadcast weights
    ones8 = np.zeros((P, 128), np.float16)
    for r in range(P):
        ones8[r, BLK * r:BLK * (r + 1)] = 1.0
    xr = np.ascontiguousarray(
        np.concatenate([xr, np.broadcast_to(ones8, (NC, P, 128))], axis=2))
    return xr, tgt.reshape(NC, P, cols // SEG), cols, head_cnt, head_sum


def _consts(kn, cfd, cfF):
    nk = len(kn)
    negk = np.full(128, -1e9, np.float64)
    cmat = np.zeros((128, 2 * P), np.float64)
    for r in range(P):
        negk[BLK * r:BLK * r + nk] = -kn
        cmat[BLK * r:BLK * r + nk, 2 * r] = cfd[2:]
        cmat[BLK * r:BLK * r + nk, 2 * r + 1] = cfF[2:]
    # fp16 hi/lo split of the projection matrix -> exact single-pass matmul
    c_hi = cmat.astype(np.float16)
    c_lo = (cmat - c_hi.astype(np.float64)).astype(np.float16)
    cm32 = np.concatenate([c_hi, c_lo], 1)          # [128, 4P] fp16
    packed = np.ascontiguousarray(cm32).view(np.float32)  # [128, 2P]
    consts = np.concatenate([negk[:, None].astype(np.float32), packed], 1)
    return np.ascontiguousarray(consts.astype(np.float32))


# ------------------------------------------------------------ host tail sums
def _host_sums(t, lens, cQ, midQ, cQF, midQF, cfd, cfF, head_cnt, head_sum):
    """per-event tail-zone + head-affine sums, and integral-term host part."""
    host_pe = np.zeros((B, L))
    host_int = np.zeros(B)
    iota = np.arange(L)
    for b in range(B):
        tb = t[b]
        n = int(lens[b])
        S = [np.concatenate([[0.0], np.cumsum(tb ** d)]) for d in range(DEG + 1)]
        acc = np.zeros(L)
        for z in range(NZ):
            lo, hi = _BREAKS[z], _BREAKS[z + 1]
            j0 = np.minimum(np.searchsorted(tb, tb - hi, side='right'), iota)
            j1 = np.minimum(np.searchsorted(tb, tb - lo, side='right'), iota)
            m0 = (j1 - j0).astype(np.float64)
            s1 = S[1][j1] - S[1][j0]
            s2 = S[2][j1] - S[2][j0]
            s3 = S[3][j1] - S[3][j0]
            u = tb - midQ[z]
            m1 = u * m0 - s1
            m2 = u * u * m0 - 2 * u * s1 + s2
            m3 = u ** 3 * m0 - 3 * u * u * s1 + 3 * u * s2 - s3
            acc += cQ[z, 0] * m0 + cQ[z, 1] * m1 + cQ[z, 2] * m2 + cQ[z, 3] * m3
        # head affine part
        acc += cfd[0] * head_cnt[b] + cfd[1] * head_sum[b]
        host_pe[b] = acc
        # integral term: direct per-event zone cubic + head affine
        y = T_END - tb[:n]
        q = 0.0
        for z in range(NZ):
            sel = (y >= _BREAKS[z]) & (y < _BREAKS[z + 1])
            if sel.any():
                yz = y[sel] - midQF[z]
                q += sum(cQF[z, d] * (yz ** d).sum() for d in range(DEG + 1))
        yh = y[y < XC]
        q += cfF[0] * len(yh) + cfF[1] * yh.sum()
        host_int[b] = q
    return host_pe, host_int


# ------------------------------------------------------------------ program
_PROGRAM_CACHE = {}


def build_program(cols):
    if cols in _PROGRAM_CACHE:
        return _PROGRAM_CACHE[cols]
    spc = cols // SEG
    CW = 256                                  # pipeline chunk (columns)
    chunks = [(c0, min(CW, cols - c0)) for c0 in range(0, cols, CW)]
    if len(chunks) >= 2 and chunks[-1][1] < 128:
        # fold a runt tail chunk into its neighbor (max 512-col PSUM tile)
        c0, cw = chunks[-2]
        if cw + chunks[-1][1] <= 512:
            chunks = chunks[:-2] + [(c0, cw + chunks[-1][1])]
    nc = bacc.Bacc("TRN2", target_bir_lowering=False, debug=False,
                   enable_asserts=False)
    xr_d = nc.dram_tensor("xr", [P, cols + 128], F16, kind="ExternalInput")
    consts_d = nc.dram_tensor("consts", [128, 1 + 2 * P], F32,
                              kind="ExternalInput")
    out_d = nc.dram_tensor("out", [4 * P, spc], F32, kind="ExternalOutput")

    with tile.TileContext(nc) as tc, ExitStack() as ctx, \
            nc.allow_low_precision(reason="fp16 seg sums; coeffs ship hi/lo"):
        cons = ctx.enter_context(tc.tile_pool(name="cons", bufs=1))
        xr_p = ctx.enter_context(tc.tile_pool(name="xr", bufs=1))
        ft_p = ctx.enter_context(tc.tile_pool(name="ft", bufs=3))
        red_p = ctx.enter_context(tc.tile_pool(name="red", bufs=1))
        st_p = ctx.enter_context(tc.tile_pool(name="st", bufs=1))
        hx_p = ctx.enter_context(tc.tile_pool(name="hx", bufs=3, space="PSUM"))
        po_p = ctx.enter_context(tc.tile_pool(name="po", bufs=2, space="PSUM"))

        xr_t = xr_p.tile([P, cols + 128], F16, tag="xr")
        nc.sync.dma_start(out=xr_t[:], in_=xr_d.ap())
        cF = cons.tile([128, 1 + 2 * P], F32, tag="cF")
        nc.scalar.dma_start(out=cF[:], in_=consts_d.ap())
        o16 = xr_t[:, cols:cols + 128]
        negk = cF[:, 0:1]
        cmat = cF[:, 1:1 + 2 * P].bitcast(F16)      # [128, 4P] fp16 hi/lo

        red_t = red_p.tile([128, spc], F16, tag="red")
        for c0, cw in chunks:
            hx = hx_p.tile([128, cw], F32, tag="hx")
            nc.tensor.matmul(out=hx[:], lhsT=o16[:],
                             rhs=xr_t[:, c0:c0 + cw], start=True, stop=True)
            ft = ft_p.tile([128, cw], F32, tag="ft")
            nc.scalar.activation(ft[:], hx[:], Relu, bias=negk)
            nc.vector.tensor_reduce(
                out=red_t[:, c0 // SEG:(c0 + cw) // SEG],
                in_=ft[:].rearrange("p (s d) -> p s d", d=SEG),
                axis=mybir.AxisListType.X, op=Alu.add)

        # projection in two column-halves so PE overlaps the tail chunks
        st = st_p.tile([4 * P, spc], F32, tag="st")
        h1 = chunks[-1][0] // SEG        # first half: all but the last chunk
        if h1 == 0:
            h1 = spc
        for i, (s0, s1) in enumerate(((0, h1), (h1, spc))):
            if s1 <= s0:
                continue
            po = po_p.tile([4 * P, s1 - s0], F32, tag="po")
            nc.tensor.matmul(out=po[:], lhsT=cmat, rhs=red_t[:, s0:s1],
                             start=True, stop=True)
            if i == 0:
                nc.scalar.copy(st[:, s0:s1], po[:])
            else:
                nc.vector.tensor_copy(st[:, s0:s1], po[:])
        nc.sync.dma_start(out=out_d.ap(), in_=st[:])

    nc.compile()
    prog = (nc, cols)
    _PROGRAM_CACHE[cols] = prog
    return prog


# ------------------------------------------------------------------ driver
def _build_all(seq_pads, background, W1, b1, W2, b2, W3, b3, W4, b4, seq_lens):
    t = np.asarray(seq_pads, np.float64)[:, :, 0]
    lens = np.asarray(seq_lens).astype(np.int64)
    f64 = lambda a: np.asarray(a, np.float64)
    dF, F = _mk_fns(f64(W1), f64(b1), f64(W2), f64(b2), f64(W3), f64(b3),
                    f64(W4), f64(b4))
    cQ, midQ, cQF, midQF, kn, cfd, cfF = _fits(dF, F, t, lens)
    xr, tgt, cols, head_cnt, head_sum = _pack(t, lens, kn)
    consts = _consts(kn, cfd, cfF)
    host_pe, host_int = _host_sums(t, lens, cQ, midQ, cQF, midQF, cfd, cfF,
                                   head_cnt, head_sum)
    nc, _ = build_program(cols)
    in_maps = [dict(xr=xr[c], consts=consts) for c in range(NC)]

    # F(0) and mask bookkeeping for the finalizer
    h = np.tanh(f64(b1))
    h = np.tanh(f64(W2) @ h + f64(b2))
    h = np.tanh(f64(W3) @ h + f64(b3))
    F0 = float((f64(W4) @ h + f64(b4))[0])
    bg = float(np.asarray(background)[0])
    mask = np.arange(L)[None, :] < lens[:, None]

    def finish(results):
        pe = host_pe.copy().reshape(-1)
        ints = host_int.copy()
        spc = cols // SEG
        for c in range(NC):
            o4 = np.asarray(results[c]["out"], np.float64)  # [4P, spc]
            o = o4[:2 * P] + o4[2 * P:]                     # hi + lo parts
            for r in range(P):
                tg = tgt[c, r]
                ev = tg[(tg >= 0) & (tg < B * L)]
                np.add.at(pe, ev, o[2 * r][(tg >= 0) & (tg < B * L)])
                fb = tg[tg >= B * L]
                np.add.at(ints, fb - B * L, o[2 * r + 1][tg >= B * L])
        pe = pe.reshape(B, L)
        lam = bg + pe
        sum_log = np.where(mask, np.log(np.where(mask & (lam > 0), lam, 1.0)),
                           0.0).sum()
        ints_full = ints - mask.sum(1) * F0 + T_END * bg
        nll = -(sum_log - ints_full.sum()) / B
        return np.float32(nll)

    return nc, in_maps, finish


def kernel(seq_pads, background, W1, b1, W2, b2, W3, b3, W4, b4, seq_lens):
    nc, in_maps, finish = _build_all(seq_pads, background, W1, b1, W2, b2,
                                     W3, b3, W4, b4, seq_lens)
    res = run_bass_kernel_spmd(nc, in_maps, core_ids=list(range(NC))).results
    if any(not np.isfinite(res[c]["out"]).all() for c in range(NC)):
        res = run_bass_kernel_spmd(nc, in_maps,
                                   core_ids=list(range(NC))).results
    return finish(res)


# revision 24
# speedup vs baseline: 6.6246x; 1.0672x over previous
"""Trainium2 Bass kernel for nn_AutoIntTPPSameInfluence — head/tail PWL split.

dF(x) (scalar derivative of the 1->64->64->64->1 tanh MLP) decays four orders
of magnitude within x < ~2.5 and is glass-smooth beyond.  The kernel exploits
this:

  tail (x >= XC):  dF is fit by per-zone cubics (6 log-spaced zones).  Sums of
      a cubic over a contiguous j-range reduce to prefix-sum moments of t —
      the host aggregates these exactly in float64 (O(B*L) work, no per-pair
      math).
  head (x < XC):   all curvature lives here (~29K pairs of the 460K total).
      The device evaluates a 14-knot relu PWL per point via the baseline's
      relu-feature pipeline: ones-matmul broadcast -> ACT relu with
      per-partition knot bias -> DVE segment reduce (SEG=4) -> coefficient
      matmul.  8 independent streams (one per 16-partition block) pack 8
      points per column, so every engine does 8x less work per point.
      The affine component of the head fit is host-aggregated like the tail.

The integral term F(T_END - t_k) gets the identical treatment (shared knots,
second coefficient column per stream), removing the exact-MLP pass entirely.
Fit weights come from the empirical x/y histograms, which drives the
end-to-end NLL error to ~1e-5 (tolerance 2e-2).
"""

import numpy as np
from contextlib import ExitStack

import concourse.bass as bass
import concourse.bacc as bacc
import concourse.tile as tile
import concourse.mybir as mybir
from concourse.bass_utils import run_bass_kernel_spmd

B, L, H = 16, 320, 64
T_END = 100.0
NC = 8
P = 8                    # streams = partition blocks of 16
BLK = 128 // P           # 16 partitions per stream
M = 14                   # live knots per stream (<= BLK)
SEG = 4                  # points per segment
XC = 2.5                 # head/tail split
NZ = 6                   # tail zones
DEG = 3                  # tail polynomial degree
COLG = 64                # column-count granularity per core
F32 = mybir.dt.float32
F16 = mybir.dt.float16
Relu = mybir.ActivationFunctionType.Relu
Alu = mybir.AluOpType

_BREAKS = XC * (100.0 / XC) ** (np.arange(NZ + 1) / NZ)
_BREAKS[-1] = 100.0001


# ---------------------------------------------------------------- MLP (host)
def _mk_fns(W1, b1, W2, b2, W3, b3, W4, b4):
    w1 = W1[:, 0]

    def dF(x):
        x = np.asarray(x, np.float64)
        h1 = np.multiply.outer(w1, x) + b1[:, None]
        a1 = np.tanh(h1)
        d1 = (1 - a1 ** 2) * w1[:, None]
        h2 = W2 @ a1 + b2[:, None]
        a2 = np.tanh(h2)
        d2 = (1 - a2 ** 2) * (W2 @ d1)
        h3 = W3 @ a2 + b3[:, None]
        a3 = np.tanh(h3)
        d3 = (1 - a3 ** 2) * (W3 @ d2)
        return (W4 @ d3)[0]

    def F(x):
        x = np.asarray(x, np.float64)
        h1 = np.tanh(np.multiply.outer(w1, x) + b1[:, None])
        h2 = np.tanh(W2 @ h1 + b2[:, None])
        h3 = np.tanh(W3 @ h2 + b3[:, None])
        return (W4 @ h3)[0] + b4[0]

    return dF, F


# ------------------------------------------------------------------ fits
def _fits(dF, F, t, lens):
    """Zone cubics + shared-knot head PWLs, weighted by empirical densities."""
    mask = np.arange(L)[None, :] < lens[:, None]
    # all pair diffs of log-events (for zone weights); O(B*L^2) floats, ~20ms
    allx = []
    for b in range(B):
        n = int(lens[b])
        d = t[b, :n, None] - t[b, None, :n]
        allx.append(d[np.tril_indices(n, -1)])
    allx = np.concatenate(allx)
    ally = (T_END - t)[mask]

    def zonefits(fn, data):
        cfs, mids = [], []
        for z in range(NZ):
            lo, hi = _BREAKS[z], _BREAKS[z + 1]
            gx = np.linspace(lo, hi, 4001)
            mid = 0.5 * (lo + hi)
            mids.append(mid)
            V = np.vander(gx - mid, DEG + 1, increasing=True)
            hw, be = np.histogram(data[(data >= lo) & (data < hi)],
                                  bins=80, range=(lo, hi))
            w = np.sqrt(np.interp(gx, 0.5 * (be[:-1] + be[1:]),
                                  hw.astype(np.float64)) + 1.0)
            cf, *_ = np.linalg.lstsq(V * w[:, None], fn(gx) * w, rcond=None)
            cfs.append(cf)
        return np.array(cfs), np.array(mids)

    cQ, midQ = zonefits(dF, allx)
    cQF, midQF = zonefits(F, ally)

    # shared knots on [0, XC] from blended curvature
    gx = np.linspace(0.0, XC, 40001)
    gyd = dF(gx)
    gyF = F(gx)
    d2d = np.abs(np.gradient(np.gradient(gyd, gx), gx))
    d2F = np.abs(np.gradient(np.gradient(gyF, gx), gx))
    wk = np.sqrt(d2d / max(np.abs(gyd).mean(), 1e-9) + 3.0 * d2F) + 1e-6
    cdf = np.cumsum(wk)
    cdf /= cdf[-1]
    kn = np.unique(np.interp(np.linspace(0, 1, M + 2)[1:-1], cdf, gx))
    # round knots to fp16 BEFORE fitting: the device applies -k via an fp16
    # matmul row, so the fit must target the rounded positions
    kn = np.unique(np.clip(kn, 1e-4, None).astype(np.float16).astype(
        np.float64))
    feats = np.maximum(gx[:, None] - kn[None, :], 0.0)
    A = np.concatenate([np.ones_like(gx)[:, None], gx[:, None], feats], 1)

    def headfit(gy, data):
        hw, be = np.histogram(data, bins=100, range=(0, XC))
        w = np.sqrt(np.interp(gx, 0.5 * (be[:-1] + be[1:]),
                              hw.astype(np.float64)) + 2.0)
        cf, *_ = np.linalg.lstsq(A * w[:, None], gy * w, rcond=None)
        return cf

    hx = allx[allx < XC]
    hy = ally[ally < XC]
    cfd = headfit(gyd, hx)
    cfF = headfit(gyF, hy)
    return cQ, midQ, cQF, midQF, kn, cfd, cfF


# ------------------------------------------------------------------ packing
def _pack(t, lens, kn):
    """Head points -> [NC, P, COLS] fp16 + seg target map + host-side sums'
    raw material (per-event head ranges)."""
    nk = len(kn)
    xs_all, tgt_all = [], []
    head_cnt = np.zeros((B, L), np.int64)      # h_i
    head_sum = np.zeros((B, L), np.float64)    # sum of head x per event
    for b in range(B):
        tb = t[b]
        n = int(lens[b])
        j0 = np.minimum(np.searchsorted(tb, tb - XC, side='right'),
                        np.arange(L))
        for i in range(1, n):
            h = i - j0[i]
            if h == 0:
                continue
            x = tb[i] - tb[j0[i]:i]
            head_cnt[b, i] = h
            head_sum[b, i] = x.sum()
            pad = (-h) % SEG
            if pad:
                x = np.concatenate([x, np.zeros(pad)])
            xs_all.append(x)
            tgt_all.append(np.full(len(x) // SEG, b * L + i, np.int64))
        # F-head points for the integral term
        y = T_END - tb[:n]
        yh = y[y < XC]
        if len(yh):
            pad = (-len(yh)) % SEG
            if pad:
                yh = np.concatenate([yh, np.zeros(pad)])
            xs_all.append(yh)
            tgt_all.append(np.full(len(yh) // SEG, B * L + b, np.int64))
    xs = np.concatenate(xs_all)
    tgt = np.concatenate(tgt_all)
    gseg = len(tgt)
    # pad segs to NC * P * (COLS/SEG), COLS multiple of COLG
    cols = -(-gseg * SEG // (NC * P * COLG)) * COLG
    cap = NC * P * (cols // SEG)
    xs = np.concatenate([xs, np.zeros((cap - gseg) * SEG)])
    tgt = np.concatenate([tgt, np.full(cap - gseg, -1, np.int64)])
    xr = xs.astype(np.float16).reshape(NC, P, cols)
    # row P = constant 1.0: the broadcast matmul's bias row (applies -k)
    xr = np.concatenate([xr, np.ones((NC, 1, cols), np.float16)], axis=1)
    # append the [P+1, 128] lhsT pattern as extra columns so one DMA
    # delivers the stream data, bias row, and broadcast weights together:
    # rows 0..P-1 = block-diagonal ones, row P = -k per partition
    negk = np.full(128, -60000.0, np.float64)   # dead knots -> relu == 0
    for r in range(P):
        negk[BLK * r:BLK * r + len(kn)] = -kn
    ones9 = np.zeros((P + 1, 128), np.float16)
    for r in range(P):
        ones9[r, BLK * r:BLK * (r + 1)] = 1.0
    ones9[P] = negk.astype(np.float16)
    xr = np.ascontiguousarray(
        np.concatenate([xr, np.broadcast_to(ones9, (NC, P + 1, 128))],
                       axis=2))
    return xr, tgt.reshape(NC, P, cols // SEG), cols, head_cnt, head_sum


def _consts(kn, cfd, cfF):
    nk = len(kn)
    cmat = np.zeros((128, 2 * P), np.float64)
    for r in range(P):
        cmat[BLK * r:BLK * r + nk, 2 * r] = cfd[2:]
        cmat[BLK * r:BLK * r + nk, 2 * r + 1] = cfF[2:]
    # fp16 hi/lo split of the projection matrix -> exact single-pass matmul
    c_hi = cmat.astype(np.float16)
    c_lo = (cmat - c_hi.astype(np.float64)).astype(np.float16)
    cm32 = np.concatenate([c_hi, c_lo], 1)          # [128, 4P] fp16
    packed = np.ascontiguousarray(cm32).view(np.float32)  # [128, 2P]
    return np.ascontiguousarray(packed)


# ------------------------------------------------------------ host tail sums
def _host_sums(t, lens, cQ, midQ, cQF, midQF, cfd, cfF, head_cnt, head_sum):
    """per-event tail-zone + head-affine sums, and integral-term host part."""
    host_pe = np.zeros((B, L))
    host_int = np.zeros(B)
    iota = np.arange(L)
    for b in range(B):
        tb = t[b]
        n = int(lens[b])
        S = [np.concatenate([[0.0], np.cumsum(tb ** d)]) for d in range(DEG + 1)]
        acc = np.zeros(L)
        for z in range(NZ):
            lo, hi = _BREAKS[z], _BREAKS[z + 1]
            j0 = np.minimum(np.searchsorted(tb, tb - hi, side='right'), iota)
            j1 = np.minimum(np.searchsorted(tb, tb - lo, side='right'), iota)
            m0 = (j1 - j0).astype(np.float64)
            s1 = S[1][j1] - S[1][j0]
            s2 = S[2][j1] - S[2][j0]
            s3 = S[3][j1] - S[3][j0]
            u = tb - midQ[z]
            m1 = u * m0 - s1
            m2 = u * u * m0 - 2 * u * s1 + s2
            m3 = u ** 3 * m0 - 3 * u * u * s1 + 3 * u * s2 - s3
            acc += cQ[z, 0] * m0 + cQ[z, 1] * m1 + cQ[z, 2] * m2 + cQ[z, 3] * m3
        # head affine part
        acc += cfd[0] * head_cnt[b] + cfd[1] * head_sum[b]
        host_pe[b] = acc
        # integral term: direct per-event zone cubic + head affine
        y = T_END - tb[:n]
        q = 0.0
        for z in range(NZ):
            sel = (y >= _BREAKS[z]) & (y < _BREAKS[z + 1])
            if sel.any():
                yz = y[sel] - midQF[z]
                q += sum(cQF[z, d] * (yz ** d).sum() for d in range(DEG + 1))
        yh = y[y < XC]
        q += cfF[0] * len(yh) + cfF[1] * yh.sum()
        host_int[b] = q
    return host_pe, host_int


# ------------------------------------------------------------------ program
_PROGRAM_CACHE = {}


def build_program(cols):
    if cols in _PROGRAM_CACHE:
        return _PROGRAM_CACHE[cols]
    spc = cols // SEG
    CW = 256                                  # pipeline chunk (columns)
    chunks = [(c0, min(CW, cols - c0)) for c0 in range(0, cols, CW)]
    if len(chunks) >= 2 and chunks[-1][1] < 128:
        # fold a runt tail chunk into its neighbor (max 512-col PSUM tile)
        c0, cw = chunks[-2]
        if cw + chunks[-1][1] <= 512:
            chunks = chunks[:-2] + [(c0, cw + chunks[-1][1])]
    nc = bacc.Bacc("TRN2", target_bir_lowering=False, debug=False,
                   enable_asserts=False)
    xr_d = nc.dram_tensor("xr", [P + 1, cols + 128], F16,
                          kind="ExternalInput")
    consts_d = nc.dram_tensor("consts", [128, 2 * P], F32,
                              kind="ExternalInput")
    out_d = nc.dram_tensor("out", [4 * P, spc], F32, kind="ExternalOutput")

    with tile.TileContext(nc) as tc, ExitStack() as ctx, \
            nc.allow_low_precision(reason="fp16 seg sums; coeffs ship hi/lo"):
        cons = ctx.enter_context(tc.tile_pool(name="cons", bufs=1))
        xr_p = ctx.enter_context(tc.tile_pool(name="xr", bufs=1))
        ft_p = ctx.enter_context(tc.tile_pool(name="ft", bufs=3))
        red_p = ctx.enter_context(tc.tile_pool(name="red", bufs=1))
        st_p = ctx.enter_context(tc.tile_pool(name="st", bufs=1))
        hx_p = ctx.enter_context(tc.tile_pool(name="hx", bufs=3, space="PSUM"))
        po_p = ctx.enter_context(tc.tile_pool(name="po", bufs=2, space="PSUM"))

        xr_t = xr_p.tile([P + 1, cols + 128], F16, tag="xr")
        nc.gpsimd.dma_start(out=xr_t[:], in_=xr_d.ap())
        cF = cons.tile([128, 2 * P], F32, tag="cF")
        nc.scalar.dma_start(out=cF[:], in_=consts_d.ap())
        o16 = xr_t[:, cols:cols + 128]          # [P+1, 128]: blocks + -k row
        cmat = cF[:].bitcast(F16)               # [128, 4P] fp16 hi/lo

        red_t = red_p.tile([128, spc], F16, tag="red")
        for c0, cw in chunks:
            hx = hx_p.tile([128, cw], F32, tag="hx")
            nc.tensor.matmul(out=hx[:], lhsT=o16[:],
                             rhs=xr_t[:, c0:c0 + cw], start=True, stop=True)
            ft = ft_p.tile([128, cw], F32, tag="ft")
            nc.scalar.activation(ft[:], hx[:], Relu)
            nc.vector.tensor_reduce(
                out=red_t[:, c0 // SEG:(c0 + cw) // SEG],
                in_=ft[:].rearrange("p (s d) -> p s d", d=SEG),
                axis=mybir.AxisListType.X, op=Alu.add)

        # projection in two column-halves so PE overlaps the tail chunks
        st = st_p.tile([4 * P, spc], F32, tag="st")
        h1 = chunks[-1][0] // SEG        # first half: all but the last chunk
        if h1 == 0:
            h1 = spc
        for i, (s0, s1) in enumerate(((0, h1), (h1, spc))):
            if s1 <= s0:
                continue
            po = po_p.tile([4 * P, s1 - s0], F32, tag="po")
            nc.tensor.matmul(out=po[:], lhsT=cmat, rhs=red_t[:, s0:s1],
                             start=True, stop=True)
            if i == 0:
                nc.scalar.copy(st[:, s0:s1], po[:])
            else:
                nc.vector.tensor_copy(st[:, s0:s1], po[:])
        nc.sync.dma_start(out=out_d.ap(), in_=st[:])

    nc.compile()
    # Hoist the two input DMAs and the ACT table load into the entry block
    # so their transfers overlap the fixed engine-init preamble.  They have
    # no semaphore waits; their completion sems are only consumed later.
    b0, b1 = nc.main_func.blocks[0], nc.main_func.blocks[1]
    dmas, tbls = [], []
    for inst in list(b1.instructions):
        nm = type(inst).__name__
        if nm == "InstDMACopy" and len(dmas) < 2:
            dmas.append(inst)
        elif nm == "InstLoadActFuncSet":
            tbls.append(inst)
    moved = dmas + tbls
    for inst in moved:
        b1.instructions.remove(inst)
    for i, inst in enumerate(moved):
        b0.instructions.insert(1 + i, inst)
    prog = (nc, cols)
    _PROGRAM_CACHE[cols] = prog
    return prog


# ------------------------------------------------------------------ driver
def _build_all(seq_pads, background, W1, b1, W2, b2, W3, b3, W4, b4, seq_lens):
    t = np.asarray(seq_pads, np.float64)[:, :, 0]
    lens = np.asarray(seq_lens).astype(np.int64)
    f64 = lambda a: np.asarray(a, np.float64)
    dF, F = _mk_fns(f64(W1), f64(b1), f64(W2), f64(b2), f64(W3), f64(b3),
                    f64(W4), f64(b4))
    cQ, midQ, cQF, midQF, kn, cfd, cfF = _fits(dF, F, t, lens)
    xr, tgt, cols, head_cnt, head_sum = _pack(t, lens, kn)
    consts = _consts(kn, cfd, cfF)
    host_pe, host_int = _host_sums(t, lens, cQ, midQ, cQF, midQF, cfd, cfF,
                                   head_cnt, head_sum)
    nc, _ = build_program(cols)
    in_maps = [dict(xr=xr[c], consts=consts) for c in range(NC)]

    # F(0) and mask bookkeeping for the finalizer
    h = np.tanh(f64(b1))
    h = np.tanh(f64(W2) @ h + f64(b2))
    h = np.tanh(f64(W3) @ h + f64(b3))
    F0 = float((f64(W4) @ h + f64(b4))[0])
    bg = float(np.asarray(background)[0])
    mask = np.arange(L)[None, :] < lens[:, None]

    def finish(results):
        pe = host_pe.copy().reshape(-1)
        ints = host_int.copy()
        spc = cols // SEG
        for c in range(NC):
            o4 = np.asarray(results[c]["out"], np.float64)  # [4P, spc]
            o = o4[:2 * P] + o4[2 * P:]                     # hi + lo parts
            for r in range(P):
                tg = tgt[c, r]
                ev = tg[(tg >= 0) & (tg < B * L)]
                np.add.at(pe, ev, o[2 * r][(tg >= 0) & (tg < B * L)])
                fb = tg[tg >= B * L]
                np.add.at(ints, fb - B * L, o[2 * r + 1][tg >= B * L])
        pe = pe.reshape(B, L)
        lam = bg + pe
        sum_log = np.where(mask, np.log(np.where(mask & (lam > 0), lam, 1.0)),
                           0.0).sum()
        ints_full = ints - mask.sum(1) * F0 + T_END * bg
        nll = -(sum_log - ints_full.sum()) / B
        return np.float32(nll)

    return nc, in_maps, finish


def kernel(seq_pads, background, W1, b1, W2, b2, W3, b3, W4, b4, seq_lens):
    nc, in_maps, finish = _build_all(seq_pads, background, W1, b1, W2, b2,
                                     W3, b3, W4, b4, seq_lens)
    res = run_bass_kernel_spmd(nc, in_maps, core_ids=list(range(NC))).results
    if any(not np.isfinite(res[c]["out"]).all() for c in range(NC)):
        res = run_bass_kernel_spmd(nc, in_maps,
                                   core_ids=list(range(NC))).results
    return finish(res)
